# revision 20
# baseline (speedup 1.0000x reference)
"""Trainium2 Bass kernel for nn_DifferentiableEmbedding (moe_routing).

Computation (per token t):
    data = emb_table[id]                      # (512,)
    g    = gate_table[id] * 512               # scalar in (0.512, 512)
    mask = (iota512 < g)                      # 0/1 mask (frac term ~1e-9 dropped)
    e    = clip(ceil(g) // 102, 0, 4)         # expert index
    y    = (data*mask) @ W[e].T + b[e]

Sharding: data-parallel on B (8 batch rows -> 8 cores). Tables and expert
weights replicated per core.

Key facts used:
  * count = sum(mask) = ceil(g) exactly, so expert e <=> g in (102e-1, 102e+101]
    -- indicators are computed with pure comparisons (no floor op needed).
  * tokens of expert e have mask zero beyond feature 102e+101, so expert e
    only needs the first ceil((102e+101)/128) of the 4 K-chunks: [1,2,3,4,4]
    -> 14 accumulating matmuls per 128-token tile instead of 20.
  * bias is added with one K=5 matmul: lhsT = one-hot(expert)ᵀ [5,128],
    rhs = bias [5,512].
  * matmuls run as float32r (full PE rate at N=512).
"""

import os
import sys

import numpy as np

sys.path.insert(0, "/opt/trn_rl_repo")

import concourse.bass as bass  # noqa: E402
import concourse.tile as tile  # noqa: E402
from concourse import bacc, bass_utils, mybir  # noqa: E402

VOCAB, D, B, S, E = 50257, 512, 8, 2048, 5
P = 128                     # partitions / tokens per tile
NT = S // P                 # 16 token tiles per core
NK = D // P                 # 4 contraction chunks
CHUNKS_PER_EXPERT = [1, 2, 3, 4, 4]   # tail-chunk trick
NJ = sum(CHUNKS_PER_EXPERT)           # 14 (expert, chunk) pairs
# expert-boundary thresholds in g = gate*512 space: expert e <=> g in (LO[e], HI[e]]
BOUND_LO = [-1.0, 101.0, 203.0, 305.0, 407.0]
BOUND_HI = [101.0, 203.0, 305.0, 407.0, 1e30]

F32 = mybir.dt.float32
F32R = mybir.dt.float32r
I32 = mybir.dt.int32
DA = 528  # augmented row: 512 emb + gate at col 512 + pad to 64B multiple
DG = 512  # gate column within the augmented row


def build_program(debug_taps=False):
    """Build the single-core Tile program (same program runs SPMD on 8 cores)."""
    nc = bacc.Bacc(
        "TRN2",
        target_bir_lowering=False,
        debug=False,
        enable_asserts=False,
        num_devices=8,
    )

    ids = nc.dram_tensor("ids", [P, NT], I32, kind="ExternalInput").ap()
    # augmented table: [:, :512] = emb_table, [:, 512] = gate_table (pad to 528
    # to keep rows 64B-aligned for the gather)
    emb = nc.dram_tensor("emb", [VOCAB, DA], F32, kind="ExternalInput").ap()
    wt = nc.dram_tensor("wt", [P, NJ, D], F32R, kind="ExternalInput").ap()
    bias = nc.dram_tensor("bias", [E, D], F32R, kind="ExternalInput").ap()
    iota = nc.dram_tensor("iota", [P, D], F32, kind="ExternalInput").ap()
    ident = nc.dram_tensor("ident", [P, P], F32, kind="ExternalInput").ap()
    y = nc.dram_tensor("y", [S, D], F32, kind="ExternalOutput").ap()
    if debug_taps:
        dbg_emb = nc.dram_tensor("dbg_emb", [P, NT, DA], F32, kind="ExternalOutput").ap()
        dbg_gsc = nc.dram_tensor("dbg_gsc", [P, NT], F32, kind="ExternalOutput").ap()
        dbg_ind = nc.dram_tensor("dbg_ind", [P, E, NT], F32, kind="ExternalOutput").ap()
        dbg_xm = nc.dram_tensor("dbg_xm", [P, D], F32, kind="ExternalOutput").ap()
        dbg_xmt = nc.dram_tensor("dbg_xmt", [P, P], F32R, kind="ExternalOutput").ap()

    with tile.TileContext(nc) as tc:
        with (
            tc.tile_pool(name="singles", bufs=1) as singles,
            tc.tile_pool(name="work", bufs=3) as work,
            tc.tile_pool(name="xe", bufs=7) as xe,
            tc.tile_pool(name="xmt", bufs=2 * NJ + 4) as xmt,
            tc.tile_pool(name="indt", bufs=3) as indtp,
            tc.tile_pool(name="tp_ps", bufs=4, space="PSUM") as tp_ps,
            tc.tile_pool(name="ind_ps", bufs=2, space="PSUM") as ind_ps,
            tc.tile_pool(name="y_ps", bufs=2, space="PSUM") as y_ps,
        ):
            # ---- constants / global loads ----
            iota_sb = singles.tile([P, D], F32)
            nc.sync.dma_start(out=iota_sb[:], in_=iota[:, :])
            ident_sb = singles.tile([P, P], F32)
            nc.sync.dma_start(out=ident_sb[:], in_=ident[:, :])
            wt_sb = singles.tile([P, NJ, D], F32R)
            nc.sync.dma_start(out=wt_sb[:], in_=wt[:, :, :])
            bias_sb = singles.tile([E, D], F32R)
            nc.sync.dma_start(out=bias_sb[:], in_=bias[:, :])
            ids_sb = singles.tile([P, NT], I32)
            nc.sync.dma_start(out=ids_sb[:], in_=ids[:, :])

            # gather emb+gate rows per 128-token tile (HW indirect DMA only
            # supports one index per partition): emb_sb[p, t, :] = emb[ids[p,t], :]
            emb_sb = singles.tile([P, NT, DA], F32)
            for t in range(NT):
                nc.gpsimd.indirect_dma_start(
                    out=emb_sb[:, t, :],
                    out_offset=None,
                    in_=emb[:, :],
                    in_offset=bass.IndirectOffsetOnAxis(
                        ap=ids_sb[:, t : t + 1], axis=0
                    ),
                )

            # g = gate * 512 (gate rides at column 512 of the augmented rows)
            gsc = singles.tile([P, NT], F32)
            nc.vector.tensor_scalar(
                out=gsc[:], in0=emb_sb[:, :, DG], scalar1=float(D), scalar2=None,
                op0=mybir.AluOpType.mult,
            )

            # expert indicators IND[p, e, t] = (g in (LO[e], HI[e]])
            ind = singles.tile([P, E, NT], F32)
            tmp = singles.tile([P, NT], F32)
            nc.vector.tensor_scalar(
                out=ind[:, 0, :], in0=gsc[:], scalar1=BOUND_HI[0], scalar2=None,
                op0=mybir.AluOpType.is_le,
            )
            for e in (1, 2, 3):
                nc.vector.tensor_scalar(
                    out=ind[:, e, :], in0=gsc[:], scalar1=BOUND_LO[e], scalar2=None,
                    op0=mybir.AluOpType.is_gt,
                )
                nc.vector.tensor_scalar(
                    out=tmp[:], in0=gsc[:], scalar1=BOUND_HI[e], scalar2=None,
                    op0=mybir.AluOpType.is_le,
                )
                nc.vector.tensor_tensor(
                    out=ind[:, e, :], in0=ind[:, e, :], in1=tmp[:],
                    op=mybir.AluOpType.mult,
                )
            nc.vector.tensor_scalar(
                out=ind[:, 4, :], in0=gsc[:], scalar1=BOUND_LO[4], scalar2=None,
                op0=mybir.AluOpType.is_gt,
            )
            if debug_taps:
                nc.sync.dma_start(out=dbg_emb[:, :, :], in_=emb_sb[:])
                nc.sync.dma_start(out=dbg_gsc[:, :], in_=gsc[:])
                nc.sync.dma_start(out=dbg_ind[:, :, :], in_=ind[:])

            # ---- per 128-token tile ----
            for t in range(NT):
                mask = work.tile([P, D], F32, tag="mask")
                nc.vector.tensor_scalar(
                    out=mask[:], in0=iota_sb[:], scalar1=gsc[:, t : t + 1],
                    scalar2=None, op0=mybir.AluOpType.is_lt,
                )
                xm = work.tile([P, D], F32, tag="xm")
                nc.vector.tensor_tensor(
                    out=xm[:], in0=mask[:], in1=emb_sb[:, t, :D],
                    op=mybir.AluOpType.mult,
                )
                if debug_taps and t == 0:
                    nc.sync.dma_start(out=dbg_xm[:, :], in_=xm[:])

                # one-hot(expert)ᵀ for the bias matmul: [5, 128]
                ind_psum = ind_ps.tile([E, P], F32, tag="indps")
                nc.tensor.transpose(
                    out=ind_psum[:], in_=ind[:, :, t], identity=ident_sb[:]
                )
                indT = indtp.tile([E, P], F32R, tag="indT")
                nc.vector.tensor_copy(out=indT[:], in_=ind_psum[:])

                y_psum = y_ps.tile([P, D], F32, tag="yps")
                first = True
                for e in range(E):
                    w_e = CHUNKS_PER_EXPERT[e] * P
                    # xm_e = xm * ind_e  (indicator scaling, per-partition)
                    xm_e = xe.tile([P, D], F32, tag="xe")
                    if e < 2:
                        nc.gpsimd.tensor_scalar(
                            out=xm_e[:, :w_e], in0=xm[:, :w_e],
                            scalar1=ind[:, e, t : t + 1], scalar2=None,
                            op0=mybir.AluOpType.mult,
                        )
                    else:
                        nc.scalar.activation(
                            out=xm_e[:, :w_e], in_=xm[:, :w_e],
                            func=mybir.ActivationFunctionType.Copy,
                            scale=ind[:, e, t : t + 1],
                        )
                    for k in range(CHUNKS_PER_EXPERT[e]):
                        j = sum(CHUNKS_PER_EXPERT[:e]) + k
                        tp = tp_ps.tile([P, P], F32, tag="tp")
                        nc.tensor.transpose(
                            out=tp[:], in_=xm_e[:, k * P : (k + 1) * P],
                            identity=ident_sb[:],
                        )
                        xmT = xmt.tile([P, P], F32R, tag="xmT")
                        if j % 2 == 0:
                            nc.vector.tensor_copy(out=xmT[:], in_=tp[:])
                        else:
                            nc.scalar.activation(
                                out=xmT[:], in_=tp[:],
                                func=mybir.ActivationFunctionType.Copy,
                            )
                        if debug_taps and t == 0 and j == 0:
                            nc.sync.dma_start(out=dbg_xmt[:, :], in_=xmT[:])
                        nc.tensor.matmul(
                            out=y_psum[:], lhsT=xmT[:], rhs=wt_sb[:, j, :],
                            start=first, stop=False,
                        )
                        first = False
                # bias: y[t_, :] += sum_e ind_e(t_) * bias[e, :]
                nc.tensor.matmul(
                    out=y_psum[:], lhsT=indT[:], rhs=bias_sb[:],
                    start=False, stop=True,
                )
                y_sb = work.tile([P, D], F32, tag="ysb")
                if t % 2 == 0:
                    nc.vector.tensor_copy(out=y_sb[:], in_=y_psum[:])
                else:
                    nc.scalar.activation(
                        out=y_sb[:], in_=y_psum[:],
                        func=mybir.ActivationFunctionType.Copy,
                    )
                nc.sync.dma_start(out=y[t * P : (t + 1) * P, :], in_=y_sb[:])

    nc.compile()
    return nc


def prep_core_inputs(input_ids_row, emb_table, gate_table, expert_w, expert_b,
                     aug=None):
    """Host-side layout prep for one core. input_ids_row: (S,) int."""
    ids = np.ascontiguousarray(
        input_ids_row.reshape(NT, P).T.astype(np.int32)
    )  # [P, NT]: ids[p, t] = token t*128+p
    if aug is None:
        aug = np.zeros((VOCAB, DA), dtype=np.float32)
        aug[:, :D] = emb_table
        aug[:, DG] = gate_table[:, 0]
    # wt[p, j, :] = expert_w[e].T[128k+p, :] = expert_w[e][:, 128k+p] for j=(e,k)
    wt_full = np.transpose(expert_w, (2, 0, 1)).reshape(NK, P, E, D)  # [k,p,e,o]
    cols = []
    for e in range(E):
        for k in range(CHUNKS_PER_EXPERT[e]):
            cols.append(wt_full[k, :, e, :])  # [P, D]
    wt = np.ascontiguousarray(np.stack(cols, axis=1), dtype=np.float32)  # [P,NJ,D]
    iota = np.ascontiguousarray(
        np.broadcast_to(np.arange(D, dtype=np.float32), (P, D))
    )
    ident = np.eye(P, dtype=np.float32)
    return {
        "ids": ids,
        "emb": aug,
        "wt": wt,
        "bias": np.ascontiguousarray(expert_b, dtype=np.float32),
        "iota": iota,
        "ident": ident,
    }


_CACHED_NC = None


def kernel(input_ids, emb_table, gate_table, expert_w, expert_b):
    global _CACHED_NC
    input_ids = np.asarray(input_ids)
    emb_table = np.asarray(emb_table, dtype=np.float32)
    gate_table = np.asarray(gate_table, dtype=np.float32)
    expert_w = np.asarray(expert_w, dtype=np.float32)
    expert_b = np.asarray(expert_b, dtype=np.float32)

    if _CACHED_NC is None:
        _CACHED_NC = build_program()
    nc = _CACHED_NC

    shared = None
    in_maps = []
    for c in range(B):
        m = prep_core_inputs(
            input_ids[c], emb_table, gate_table, expert_w, expert_b,
            aug=None if shared is None else shared["emb"],
        )
        if shared is None:
            shared = m
        else:
            # reuse identical replicated arrays across cores
            for k_ in ("emb", "wt", "bias", "iota", "ident"):
                m[k_] = shared[k_]
        in_maps.append(m)

    trace = bool(int(os.environ.get("BASS_KERNEL_TRACE", "0")))
    res = bass_utils.run_bass_kernel_spmd(
        nc, in_maps, core_ids=list(range(B)), trace=trace
    )
    kernel.last_result = res
    out = np.stack([res.results[c]["y"] for c in range(B)], axis=0)
    return out.astype(np.float32)


# revision 27
# speedup vs baseline: 1.2702x; 1.2702x over previous
"""Trainium2 Bass kernel for nn_DifferentiableEmbedding (moe_routing).

Computation (per token t):
    data = emb_table[id]                      # (512,)
    g    = gate_table[id] * 512               # scalar in (0.512, 512)
    mask = (iota512 < g)                      # 0/1 mask (frac term is exactly 0 in f32)
    e    = clip(ceil(g) // 102, 0, 4)         # expert index
    y    = (data*mask) @ W[e].T + b[e]

Sharding: data-parallel on B (8 batch rows -> 8 cores). Tables and expert
weights replicated per core.

Key facts used:
  * count = sum(mask) = ceil(g) exactly, so expert e <=> g in (102e-1, 102e+101]
    -- indicators are computed with pure comparisons (no floor op needed).
  * tokens of expert e have mask zero beyond feature 102e+101, so expert e
    only needs the first ceil((102e+101)/128) of the 4 K-chunks: [1,2,3,4,4]
    -> 14 accumulating matmuls per 128-token tile instead of 20.
  * bias is added with one K=5 matmul: lhsT = one-hot(expert)T [5,128],
    rhs = bias [5,512]; the one-hot transposes for ALL 16 tiles are done by a
    single [128,80] PE transpose.
  * matmuls run as float32r (full PE rate at N=512).
  * HW indirect DMA only honors one index per partition -> 16 per-tile
    gathers; the gate value rides in column 512 of a host-augmented table.
  * xm_e transposes are packed 4-per-PSUM-bank so each bank needs only one
    PSUM->SBUF cast.
"""

import os
import sys

import numpy as np

sys.path.insert(0, "/opt/trn_rl_repo")

import concourse.bass as bass  # noqa: E402
import concourse.tile as tile  # noqa: E402
from concourse import bacc, bass_utils, mybir  # noqa: E402

VOCAB, D, B, S, E = 50257, 512, 8, 2048, 5
P = 128                     # partitions / tokens per tile
NT = S // P                 # 16 token tiles per core
NK = D // P                 # 4 contraction chunks
CHUNKS_PER_EXPERT = [1, 2, 3, 4, 4]   # tail-chunk trick
NJ = sum(CHUNKS_PER_EXPERT)           # 14 (expert, chunk) pairs
# (expert, chunk) pairs in order, packed into ceil(14/4)=4 transpose banks
EK_PAIRS = [(e, k) for e in range(E) for k in range(CHUNKS_PER_EXPERT[e])]
# expert-boundary thresholds in g = gate*512 space: expert e <=> g in (LO[e], HI[e]]
BOUND_LO = [-1.0, 101.0, 203.0, 305.0, 407.0]
BOUND_HI = [101.0, 203.0, 305.0, 407.0, 1e30]

F32 = mybir.dt.float32
F32R = mybir.dt.float32r
I32 = mybir.dt.int32
DA = 528  # augmented row: 512 emb + gate at col 512 + pad to 64B multiple
DG = 512  # gate column within the augmented row


def build_program(debug_taps=False):
    """Build the single-core Tile program (same program runs SPMD on 8 cores)."""
    nc = bacc.Bacc(
        "TRN2",
        target_bir_lowering=False,
        debug=False,
        enable_asserts=False,
        num_devices=8,
    )

    ids = nc.dram_tensor("ids", [P, NT], I32, kind="ExternalInput").ap()
    # augmented table: [:, :512] = emb_table, [:, 512] = gate_table (pad to 528
    # to keep rows 64B-aligned for the gather)
    emb = nc.dram_tensor("emb", [VOCAB, DA], F32, kind="ExternalInput").ap()
    wt = nc.dram_tensor("wt", [P, NJ, D], F32R, kind="ExternalInput").ap()
    biasg = nc.dram_tensor("bias", [E, D], F32, kind="ExternalInput").ap()
    iota = nc.dram_tensor("iota", [P, D], F32, kind="ExternalInput").ap()
    ident = nc.dram_tensor("ident", [P, P], F32R, kind="ExternalInput").ap()
    ident32 = nc.dram_tensor("ident32", [P, P], F32, kind="ExternalInput").ap()
    y = nc.dram_tensor("y", [S, D], F32, kind="ExternalOutput").ap()
    if debug_taps:
        dbg_emb = nc.dram_tensor("dbg_emb", [P, NT, DA], F32, kind="ExternalOutput").ap()
        dbg_gsc = nc.dram_tensor("dbg_gsc", [P, NT], F32, kind="ExternalOutput").ap()
        dbg_ind = nc.dram_tensor("dbg_ind", [P, NT, E], F32, kind="ExternalOutput").ap()
        dbg_xm = nc.dram_tensor("dbg_xm", [P, D], F32, kind="ExternalOutput").ap()
        dbg_xmt = nc.dram_tensor("dbg_xmt", [P, P], F32R, kind="ExternalOutput").ap()

    with tile.TileContext(nc) as tc:
        with (
            tc.tile_pool(name="singles", bufs=1) as singles,
            tc.tile_pool(name="work", bufs=3) as work,
            tc.tile_pool(name="xe", bufs=7) as xe,
            tc.tile_pool(name="xmt", bufs=10) as xmt,
            tc.tile_pool(name="tp_ps", bufs=3, space="PSUM") as tp_ps,
            tc.tile_pool(name="y_ps", bufs=2, space="PSUM") as y_ps,
        ):
            # ---- constants / global loads ----
            iota_sb = singles.tile([P, D], F32)
            nc.sync.dma_start(out=iota_sb[:], in_=iota[:, :])
            ident_sb = singles.tile([P, P], F32R)
            nc.sync.dma_start(out=ident_sb[:], in_=ident[:, :])
            ident_sb32 = singles.tile([P, P], F32)
            nc.sync.dma_start(out=ident_sb32[:], in_=ident32[:, :])
            wt_sb = singles.tile([P, NJ, D], F32R)
            nc.sync.dma_start(out=wt_sb[:], in_=wt[:, :, :])
            ids_sb = singles.tile([P, NT], I32)
            nc.sync.dma_start(out=ids_sb[:], in_=ids[:, :])

            # gather emb+gate rows per 128-token tile (HW indirect DMA only
            # supports one index per partition): emb_sb[p, t, :] = emb[ids[p,t], :]
            emb_sb = singles.tile([P, NT, DA], F32)
            for t in range(NT):
                nc.gpsimd.indirect_dma_start(
                    out=emb_sb[:, t, :],
                    out_offset=None,
                    in_=emb[:, :],
                    in_offset=bass.IndirectOffsetOnAxis(
                        ap=ids_sb[:, t : t + 1], axis=0
                    ),
                )

            # g = gate * 512 (gate rides at column 512 of the augmented rows)
            gsc = singles.tile([P, NT], F32)
            nc.vector.tensor_scalar(
                out=gsc[:], in0=emb_sb[:, :, DG], scalar1=float(D), scalar2=None,
                op0=mybir.AluOpType.mult,
            )

            # expert indicators ind[p, t, e] = (g in (LO[e], HI[e]])
            ind = singles.tile([P, NT, E], F32)
            tmp = singles.tile([P, NT], F32)
            nc.vector.tensor_scalar(
                out=ind[:, :, 0], in0=gsc[:], scalar1=BOUND_HI[0], scalar2=None,
                op0=mybir.AluOpType.is_le,
            )
            for e in (1, 2, 3):
                nc.vector.tensor_scalar(
                    out=ind[:, :, e], in0=gsc[:], scalar1=BOUND_LO[e], scalar2=None,
                    op0=mybir.AluOpType.is_gt,
                )
                nc.vector.tensor_scalar(
                    out=tmp[:], in0=gsc[:], scalar1=BOUND_HI[e], scalar2=None,
                    op0=mybir.AluOpType.is_le,
                )
                nc.vector.tensor_tensor(
                    out=ind[:, :, e], in0=ind[:, :, e], in1=tmp[:],
                    op=mybir.AluOpType.mult,
                )
            nc.vector.tensor_scalar(
                out=ind[:, :, 4], in0=gsc[:], scalar1=BOUND_LO[4], scalar2=None,
                op0=mybir.AluOpType.is_gt,
            )

            # eidx as int32 for the per-tile bias gathers:
            # eidx = ind1 + 2*ind2 + 3*ind3 + 4*ind4
            eidx_f = singles.tile([P, NT], F32)
            nc.vector.tensor_scalar(
                out=eidx_f[:], in0=ind[:, :, 1], scalar1=1.0, scalar2=None,
                op0=mybir.AluOpType.mult,
            )
            for e in (2, 3, 4):
                nc.vector.tensor_scalar(
                    out=tmp[:], in0=ind[:, :, e], scalar1=float(e), scalar2=None,
                    op0=mybir.AluOpType.mult,
                )
                nc.vector.tensor_tensor(
                    out=eidx_f[:], in0=eidx_f[:], in1=tmp[:],
                    op=mybir.AluOpType.add,
                )
            eidx_i = singles.tile([P, NT], I32)
            nc.vector.tensor_copy(out=eidx_i[:], in_=eidx_f[:])
            # per-tile bias rows: bsel[p, t, :] = expert_b[eidx[p, t], :]
            bsel = singles.tile([P, NT, D], F32)
            for t in range(NT):
                nc.gpsimd.indirect_dma_start(
                    out=bsel[:, t, :],
                    out_offset=None,
                    in_=biasg[:, :],
                    in_offset=bass.IndirectOffsetOnAxis(
                        ap=eidx_i[:, t : t + 1], axis=0
                    ),
                )

            if debug_taps:
                nc.sync.dma_start(out=dbg_emb[:, :, :], in_=emb_sb[:])
                nc.sync.dma_start(out=dbg_gsc[:, :], in_=gsc[:])
                nc.sync.dma_start(out=dbg_ind[:, :, :], in_=ind[:])

            # ---- per 128-token tile ----
            for t in range(NT):
                mask = work.tile([P, D], F32, tag="mask")
                nc.vector.tensor_scalar(
                    out=mask[:], in0=iota_sb[:], scalar1=gsc[:, t : t + 1],
                    scalar2=None, op0=mybir.AluOpType.is_lt,
                )
                xm = work.tile([P, D], F32, tag="xm")
                nc.vector.tensor_tensor(
                    out=xm[:], in0=mask[:], in1=emb_sb[:, t, :D],
                    op=mybir.AluOpType.mult,
                )
                if debug_taps and t == 0:
                    nc.sync.dma_start(out=dbg_xm[:, :], in_=xm[:])

                # xm_e = xm * ind_e (indicator scaling, per-partition scalar)
                xms = []
                for e in range(E):
                    w_e = CHUNKS_PER_EXPERT[e] * P
                    xm_e = xe.tile([P, D], F32R, tag="xe")
                    if e in (0, 3):
                        nc.vector.tensor_scalar(
                            out=xm_e[:, :w_e], in0=xm[:, :w_e],
                            scalar1=ind[:, t, e : e + 1], scalar2=None,
                            op0=mybir.AluOpType.mult,
                        )
                    else:
                        nc.scalar.activation(
                            out=xm_e[:, :w_e], in_=xm[:, :w_e],
                            func=mybir.ActivationFunctionType.Copy,
                            scale=ind[:, t, e : e + 1],
                        )
                    xms.append(xm_e)

                # transpose the 14 (e,k) chunks, 4 per PSUM bank, one
                # PSUM->SBUF cast per bank
                xmt_tiles = []
                for g0 in range(0, NJ, 4):
                    group = EK_PAIRS[g0 : g0 + 4]
                    wg = len(group) * P
                    tp = tp_ps.tile([P, 4 * P], F32R, tag="tp")
                    for i, (e, k) in enumerate(group):
                        nc.tensor.matmul(
                            out=tp[:, i * P : (i + 1) * P],
                            lhsT=xms[e][:, k * P : (k + 1) * P],
                            rhs=ident_sb[:],
                            is_transpose=True,
                            start=(i == 0), stop=(i == len(group) - 1),
                        )
                    xT = xmt.tile([P, 4 * P], F32R, tag="xmT")
                    if (g0 // 4) % 2 == 0:
                        nc.vector.tensor_copy(out=xT[:, :wg], in_=tp[:, :wg])
                    else:
                        nc.scalar.activation(
                            out=xT[:, :wg], in_=tp[:, :wg],
                            func=mybir.ActivationFunctionType.Copy,
                        )
                    xmt_tiles.append(xT)
                if debug_taps and t == 0:
                    nc.sync.dma_start(out=dbg_xmt[:, :], in_=xmt_tiles[0][:, :P])

                y_psum = y_ps.tile([P, D], F32, tag="yps")
                for j in range(NJ):
                    nc.tensor.matmul(
                        out=y_psum[:],
                        lhsT=xmt_tiles[j // 4][:, (j % 4) * P : (j % 4 + 1) * P],
                        rhs=wt_sb[:, j, :],
                        start=(j == 0), stop=(j == NJ - 1),
                    )
                # y = y_psum + bias[eidx]  (bias add fused into the PSUM copy)
                y_sb = work.tile([P, D], F32, tag="ysb")
                nc.vector.tensor_tensor(
                    out=y_sb[:], in0=y_psum[:], in1=bsel[:, t, :],
                    op=mybir.AluOpType.add,
                )
                nc.sync.dma_start(out=y[t * P : (t + 1) * P, :], in_=y_sb[:])

    nc.compile()
    return nc


def prep_core_inputs(input_ids_row, emb_table, gate_table, expert_w, expert_b,
                     aug=None):
    """Host-side layout prep for one core. input_ids_row: (S,) int."""
    ids = np.ascontiguousarray(
        input_ids_row.reshape(NT, P).T.astype(np.int32)
    )  # [P, NT]: ids[p, t] = token t*128+p
    if aug is None:
        aug = np.zeros((VOCAB, DA), dtype=np.float32)
        aug[:, :D] = emb_table
        aug[:, DG] = gate_table[:, 0]
    # wt[p, j, :] = expert_w[e].T[128k+p, :] = expert_w[e][:, 128k+p] for j=(e,k)
    wt_full = np.transpose(expert_w, (2, 0, 1)).reshape(NK, P, E, D)  # [k,p,e,o]
    cols = []
    for e, k in EK_PAIRS:
        cols.append(wt_full[k, :, e, :])  # [P, D]
    wt = np.ascontiguousarray(np.stack(cols, axis=1), dtype=np.float32)  # [P,NJ,D]
    iota = np.ascontiguousarray(
        np.broadcast_to(np.arange(D, dtype=np.float32), (P, D))
    )
    ident = np.eye(P, dtype=np.float32)
    return {
        "ids": ids,
        "emb": aug,
        "wt": wt,
        "bias": np.ascontiguousarray(expert_b, dtype=np.float32),
        "iota": iota,
        "ident": ident,
        "ident32": ident,
    }


_CACHED_NC = None


def kernel(input_ids, emb_table, gate_table, expert_w, expert_b):
    global _CACHED_NC
    input_ids = np.asarray(input_ids)
    emb_table = np.asarray(emb_table, dtype=np.float32)
    gate_table = np.asarray(gate_table, dtype=np.float32)
    expert_w = np.asarray(expert_w, dtype=np.float32)
    expert_b = np.asarray(expert_b, dtype=np.float32)

    if _CACHED_NC is None:
        _CACHED_NC = build_program()
    nc = _CACHED_NC

    shared = None
    in_maps = []
    for c in range(B):
        m = prep_core_inputs(
            input_ids[c], emb_table, gate_table, expert_w, expert_b,
            aug=None if shared is None else shared["emb"],
        )
        if shared is None:
            shared = m
        else:
            # reuse identical replicated arrays across cores
            for k_ in ("emb", "wt", "bias", "iota", "ident", "ident32"):
                m[k_] = shared[k_]
        in_maps.append(m)

    trace = bool(int(os.environ.get("BASS_KERNEL_TRACE", "0")))
    res = bass_utils.run_bass_kernel_spmd(
        nc, in_maps, core_ids=list(range(B)), trace=trace
    )
    kernel.last_result = res
    out = np.stack([res.results[c]["y"] for c in range(B)], axis=0)
    return out.astype(np.float32)


# revision 28
# speedup vs baseline: 1.2798x; 1.0076x over previous
"""Trainium2 Bass kernel for nn_DifferentiableEmbedding (moe_routing).

Computation (per token t):
    data = emb_table[id]                      # (512,)
    g    = gate_table[id] * 512               # scalar in (0.512, 512)
    mask = (iota512 < g)                      # 0/1 mask (frac term is exactly 0 in f32)
    e    = clip(ceil(g) // 102, 0, 4)         # expert index
    y    = (data*mask) @ W[e].T + b[e]

Sharding: data-parallel on B (8 batch rows -> 8 cores). Tables and expert
weights replicated per core.

Key facts used:
  * count = sum(mask) = ceil(g) exactly, so expert e <=> g in (102e-1, 102e+101]
    -- indicators are computed with pure comparisons (no floor op needed).
  * tokens of expert e have mask zero beyond feature 102e+101, so expert e
    only needs the first ceil((102e+101)/128) of the 4 K-chunks: [1,2,3,4,4]
    -> 14 accumulating matmuls per 128-token tile instead of 20.
  * bias is added with one K=5 matmul: lhsT = one-hot(expert)T [5,128],
    rhs = bias [5,512]; the one-hot transposes for ALL 16 tiles are done by a
    single [128,80] PE transpose.
  * matmuls run as float32r (full PE rate at N=512).
  * HW indirect DMA only honors one index per partition -> 16 per-tile
    gathers; the gate value rides in column 512 of a host-augmented table.
  * xm_e transposes are packed 4-per-PSUM-bank so each bank needs only one
    PSUM->SBUF cast.
"""

import os
import sys

import numpy as np

sys.path.insert(0, "/opt/trn_rl_repo")

import concourse.bass as bass  # noqa: E402
import concourse.tile as tile  # noqa: E402
from concourse import bacc, bass_utils, mybir  # noqa: E402

VOCAB, D, B, S, E = 50257, 512, 8, 2048, 5
P = 128                     # partitions / tokens per tile
NT = S // P                 # 16 token tiles per core
NK = D // P                 # 4 contraction chunks
CHUNKS_PER_EXPERT = [1, 2, 3, 4, 4]   # tail-chunk trick
NJ = sum(CHUNKS_PER_EXPERT)           # 14 (expert, chunk) pairs
# (expert, chunk) pairs in order, packed into ceil(14/4)=4 transpose banks
EK_PAIRS = [(e, k) for e in range(E) for k in range(CHUNKS_PER_EXPERT[e])]
# expert-boundary thresholds in g = gate*512 space: expert e <=> g in (LO[e], HI[e]]
BOUND_LO = [-1.0, 101.0, 203.0, 305.0, 407.0]
BOUND_HI = [101.0, 203.0, 305.0, 407.0, 1e30]

F32 = mybir.dt.float32
F32R = mybir.dt.float32r
I32 = mybir.dt.int32
DA = 528  # augmented row: 512 emb + gate at col 512 + pad to 64B multiple
DG = 512  # gate column within the augmented row


def build_program(debug_taps=False):
    """Build the single-core Tile program (same program runs SPMD on 8 cores)."""
    nc = bacc.Bacc(
        "TRN2",
        target_bir_lowering=False,
        debug=False,
        enable_asserts=False,
        num_devices=8,
    )

    ids = nc.dram_tensor("ids", [P, NT], I32, kind="ExternalInput").ap()
    # augmented table: [:, :512] = emb_table, [:, 512] = gate_table (pad to 528
    # to keep rows 64B-aligned for the gather)
    emb = nc.dram_tensor("emb", [VOCAB, DA], F32, kind="ExternalInput").ap()
    wt = nc.dram_tensor("wt", [P, NJ, D], F32R, kind="ExternalInput").ap()
    biasg = nc.dram_tensor("bias", [E, D], F32, kind="ExternalInput").ap()
    iota = nc.dram_tensor("iota", [P, D], F32, kind="ExternalInput").ap()
    ident = nc.dram_tensor("ident", [P, P], F32R, kind="ExternalInput").ap()
    ident32 = nc.dram_tensor("ident32", [P, P], F32, kind="ExternalInput").ap()
    y = nc.dram_tensor("y", [S, D], F32, kind="ExternalOutput").ap()
    if debug_taps:
        dbg_emb = nc.dram_tensor("dbg_emb", [P, NT, DA], F32, kind="ExternalOutput").ap()
        dbg_gsc = nc.dram_tensor("dbg_gsc", [P, NT], F32, kind="ExternalOutput").ap()
        dbg_ind = nc.dram_tensor("dbg_ind", [P, NT, E], F32, kind="ExternalOutput").ap()
        dbg_xm = nc.dram_tensor("dbg_xm", [P, D], F32, kind="ExternalOutput").ap()
        dbg_xmt = nc.dram_tensor("dbg_xmt", [P, P], F32R, kind="ExternalOutput").ap()

    with tile.TileContext(nc) as tc:
        with (
            tc.tile_pool(name="singles", bufs=1) as singles,
            tc.tile_pool(name="work", bufs=4) as work,
            tc.tile_pool(name="xe", bufs=12) as xe,
            tc.tile_pool(name="xmt", bufs=12) as xmt,
            tc.tile_pool(name="tp_ps", bufs=4, space="PSUM") as tp_ps,
            tc.tile_pool(name="y_ps", bufs=3, space="PSUM") as y_ps,
        ):
            # ---- constants / global loads ----
            iota_sb = singles.tile([P, D], F32)
            nc.sync.dma_start(out=iota_sb[:], in_=iota[:, :])
            ident_sb = singles.tile([P, P], F32R)
            nc.sync.dma_start(out=ident_sb[:], in_=ident[:, :])
            ident_sb32 = singles.tile([P, P], F32)
            nc.sync.dma_start(out=ident_sb32[:], in_=ident32[:, :])
            wt_sb = singles.tile([P, NJ, D], F32R)
            nc.sync.dma_start(out=wt_sb[:], in_=wt[:, :, :])
            ids_sb = singles.tile([P, NT], I32)
            nc.sync.dma_start(out=ids_sb[:], in_=ids[:, :])

            # gather emb+gate rows per 128-token tile (HW indirect DMA only
            # supports one index per partition): emb_sb[p, t, :] = emb[ids[p,t], :]
            emb_sb = singles.tile([P, NT, DA], F32)
            for t in range(NT):
                nc.gpsimd.indirect_dma_start(
                    out=emb_sb[:, t, :],
                    out_offset=None,
                    in_=emb[:, :],
                    in_offset=bass.IndirectOffsetOnAxis(
                        ap=ids_sb[:, t : t + 1], axis=0
                    ),
                )

            # g = gate * 512 (gate rides at column 512 of the augmented rows)
            gsc = singles.tile([P, NT], F32)
            nc.vector.tensor_scalar(
                out=gsc[:], in0=emb_sb[:, :, DG], scalar1=float(D), scalar2=None,
                op0=mybir.AluOpType.mult,
            )

            # expert indicators ind[p, t, e] = (g in (LO[e], HI[e]])
            ind = singles.tile([P, NT, E], F32)
            tmp = singles.tile([P, NT], F32)
            nc.vector.tensor_scalar(
                out=ind[:, :, 0], in0=gsc[:], scalar1=BOUND_HI[0], scalar2=None,
                op0=mybir.AluOpType.is_le,
            )
            for e in (1, 2, 3):
                nc.vector.tensor_scalar(
                    out=ind[:, :, e], in0=gsc[:], scalar1=BOUND_LO[e], scalar2=None,
                    op0=mybir.AluOpType.is_gt,
                )
                nc.vector.tensor_scalar(
                    out=tmp[:], in0=gsc[:], scalar1=BOUND_HI[e], scalar2=None,
                    op0=mybir.AluOpType.is_le,
                )
                nc.vector.tensor_tensor(
                    out=ind[:, :, e], in0=ind[:, :, e], in1=tmp[:],
                    op=mybir.AluOpType.mult,
                )
            nc.vector.tensor_scalar(
                out=ind[:, :, 4], in0=gsc[:], scalar1=BOUND_LO[4], scalar2=None,
                op0=mybir.AluOpType.is_gt,
            )

            # eidx as int32 for the per-tile bias gathers:
            # eidx = ind1 + 2*ind2 + 3*ind3 + 4*ind4
            eidx_f = singles.tile([P, NT], F32)
            nc.vector.tensor_scalar(
                out=eidx_f[:], in0=ind[:, :, 1], scalar1=1.0, scalar2=None,
                op0=mybir.AluOpType.mult,
            )
            for e in (2, 3, 4):
                nc.vector.tensor_scalar(
                    out=tmp[:], in0=ind[:, :, e], scalar1=float(e), scalar2=None,
                    op0=mybir.AluOpType.mult,
                )
                nc.vector.tensor_tensor(
                    out=eidx_f[:], in0=eidx_f[:], in1=tmp[:],
                    op=mybir.AluOpType.add,
                )
            eidx_i = singles.tile([P, NT], I32)
            nc.vector.tensor_copy(out=eidx_i[:], in_=eidx_f[:])
            # per-tile bias rows: bsel[p, t, :] = expert_b[eidx[p, t], :]
            bsel = singles.tile([P, NT, D], F32)
            for t in range(NT):
                nc.gpsimd.indirect_dma_start(
                    out=bsel[:, t, :],
                    out_offset=None,
                    in_=biasg[:, :],
                    in_offset=bass.IndirectOffsetOnAxis(
                        ap=eidx_i[:, t : t + 1], axis=0
                    ),
                )

            if debug_taps:
                nc.sync.dma_start(out=dbg_emb[:, :, :], in_=emb_sb[:])
                nc.sync.dma_start(out=dbg_gsc[:, :], in_=gsc[:])
                nc.sync.dma_start(out=dbg_ind[:, :, :], in_=ind[:])

            # ---- per 128-token tile ----
            for t in range(NT):
                mask = work.tile([P, D], F32, tag="mask")
                nc.vector.tensor_scalar(
                    out=mask[:], in0=iota_sb[:], scalar1=gsc[:, t : t + 1],
                    scalar2=None, op0=mybir.AluOpType.is_lt,
                )
                xm = work.tile([P, D], F32, tag="xm")
                nc.vector.tensor_tensor(
                    out=xm[:], in0=mask[:], in1=emb_sb[:, t, :D],
                    op=mybir.AluOpType.mult,
                )
                if debug_taps and t == 0:
                    nc.sync.dma_start(out=dbg_xm[:, :], in_=xm[:])

                # xm_e = xm * ind_e (indicator scaling, per-partition scalar)
                xms = []
                for e in range(E):
                    w_e = CHUNKS_PER_EXPERT[e] * P
                    xm_e = xe.tile([P, D], F32R, tag="xe")
                    if e in (0, 3):
                        nc.vector.tensor_scalar(
                            out=xm_e[:, :w_e], in0=xm[:, :w_e],
                            scalar1=ind[:, t, e : e + 1], scalar2=None,
                            op0=mybir.AluOpType.mult,
                        )
                    else:
                        nc.scalar.activation(
                            out=xm_e[:, :w_e], in_=xm[:, :w_e],
                            func=mybir.ActivationFunctionType.Copy,
                            scale=ind[:, t, e : e + 1],
                        )
                    xms.append(xm_e)

                # transpose the 14 (e,k) chunks, 4 per PSUM bank, one
                # PSUM->SBUF cast per bank
                xmt_tiles = []
                for g0 in range(0, NJ, 4):
                    group = EK_PAIRS[g0 : g0 + 4]
                    wg = len(group) * P
                    tp = tp_ps.tile([P, 4 * P], F32R, tag="tp")
                    for i, (e, k) in enumerate(group):
                        nc.tensor.matmul(
                            out=tp[:, i * P : (i + 1) * P],
                            lhsT=xms[e][:, k * P : (k + 1) * P],
                            rhs=ident_sb[:],
                            is_transpose=True,
                            start=(i == 0), stop=(i == len(group) - 1),
                        )
                    xT = xmt.tile([P, 4 * P], F32R, tag="xmT")
                    if g0 == 0:
                        nc.vector.tensor_copy(out=xT[:, :wg], in_=tp[:, :wg])
                    else:
                        nc.scalar.activation(
                            out=xT[:, :wg], in_=tp[:, :wg],
                            func=mybir.ActivationFunctionType.Copy,
                        )
                    xmt_tiles.append(xT)
                if debug_taps and t == 0:
                    nc.sync.dma_start(out=dbg_xmt[:, :], in_=xmt_tiles[0][:, :P])

                y_psum = y_ps.tile([P, D], F32, tag="yps")
                for j in range(NJ):
                    nc.tensor.matmul(
                        out=y_psum[:],
                        lhsT=xmt_tiles[j // 4][:, (j % 4) * P : (j % 4 + 1) * P],
                        rhs=wt_sb[:, j, :],
                        start=(j == 0), stop=(j == NJ - 1),
                    )
                # y = y_psum + bias[eidx]  (bias add fused into the PSUM copy)
                y_sb = work.tile([P, D], F32, tag="ysb")
                nc.vector.tensor_tensor(
                    out=y_sb[:], in0=y_psum[:], in1=bsel[:, t, :],
                    op=mybir.AluOpType.add,
                )
                nc.sync.dma_start(out=y[t * P : (t + 1) * P, :], in_=y_sb[:])

    nc.compile()
    return nc


def prep_core_inputs(input_ids_row, emb_table, gate_table, expert_w, expert_b,
                     aug=None):
    """Host-side layout prep for one core. input_ids_row: (S,) int."""
    ids = np.ascontiguousarray(
        input_ids_row.reshape(NT, P).T.astype(np.int32)
    )  # [P, NT]: ids[p, t] = token t*128+p
    if aug is None:
        aug = np.zeros((VOCAB, DA), dtype=np.float32)
        aug[:, :D] = emb_table
        aug[:, DG] = gate_table[:, 0]
    # wt[p, j, :] = expert_w[e].T[128k+p, :] = expert_w[e][:, 128k+p] for j=(e,k)
    wt_full = np.transpose(expert_w, (2, 0, 1)).reshape(NK, P, E, D)  # [k,p,e,o]
    cols = []
    for e, k in EK_PAIRS:
        cols.append(wt_full[k, :, e, :])  # [P, D]
    wt = np.ascontiguousarray(np.stack(cols, axis=1), dtype=np.float32)  # [P,NJ,D]
    iota = np.ascontiguousarray(
        np.broadcast_to(np.arange(D, dtype=np.float32), (P, D))
    )
    ident = np.eye(P, dtype=np.float32)
    return {
        "ids": ids,
        "emb": aug,
        "wt": wt,
        "bias": np.ascontiguousarray(expert_b, dtype=np.float32),
        "iota": iota,
        "ident": ident,
        "ident32": ident,
    }


_CACHED_NC = None


def kernel(input_ids, emb_table, gate_table, expert_w, expert_b):
    global _CACHED_NC
    input_ids = np.asarray(input_ids)
    emb_table = np.asarray(emb_table, dtype=np.float32)
    gate_table = np.asarray(gate_table, dtype=np.float32)
    expert_w = np.asarray(expert_w, dtype=np.float32)
    expert_b = np.asarray(expert_b, dtype=np.float32)

    if _CACHED_NC is None:
        _CACHED_NC = build_program()
    nc = _CACHED_NC

    shared = None
    in_maps = []
    for c in range(B):
        m = prep_core_inputs(
            input_ids[c], emb_table, gate_table, expert_w, expert_b,
            aug=None if shared is None else shared["emb"],
        )
        if shared is None:
            shared = m
        else:
            # reuse identical replicated arrays across cores
            for k_ in ("emb", "wt", "bias", "iota", "ident", "ident32"):
                m[k_] = shared[k_]
        in_maps.append(m)

    trace = bool(int(os.environ.get("BASS_KERNEL_TRACE", "0")))
    res = bass_utils.run_bass_kernel_spmd(
        nc, in_maps, core_ids=list(range(B)), trace=trace
    )
    kernel.last_result = res
    out = np.stack([res.results[c]["y"] for c in range(B)], axis=0)
    return out.astype(np.float32)


# revision 31
# speedup vs baseline: 1.3849x; 1.0821x over previous
"""Trainium2 Bass kernel for nn_DifferentiableEmbedding (moe_routing).

Computation (per token t):
    data = emb_table[id]                      # (512,)
    g    = gate_table[id] * 512               # scalar in (0.512, 512)
    mask = (iota512 < g)                      # 0/1 mask (frac term is exactly 0 in f32)
    e    = clip(ceil(g) // 102, 0, 4)         # expert index
    y    = (data*mask) @ W[e].T + b[e]

Sharding: data-parallel on B (8 batch rows -> 8 cores). Tables and expert
weights replicated per core.

Key facts used:
  * count = sum(mask) = ceil(g) exactly, so expert e <=> g in (102e-1, 102e+101]
    -- indicators are computed with pure comparisons (no floor op needed).
  * tokens of expert e have mask zero beyond feature 102e+101, so expert e
    only needs the first ceil((102e+101)/128) of the 4 K-chunks: [1,2,3,4,4]
    -> 14 accumulating matmuls per 128-token tile instead of 20.
  * bias is added with one K=5 matmul: lhsT = one-hot(expert)T [5,128],
    rhs = bias [5,512]; the one-hot transposes for ALL 16 tiles are done by a
    single [128,80] PE transpose.
  * matmuls run as float32r (full PE rate at N=512).
  * HW indirect DMA only honors one index per partition -> 16 per-tile
    gathers; the gate value rides in column 512 of a host-augmented table.
  * xm_e transposes are packed 4-per-PSUM-bank so each bank needs only one
    PSUM->SBUF cast.
"""

import os
import sys

import numpy as np

sys.path.insert(0, "/opt/trn_rl_repo")

import concourse.bass as bass  # noqa: E402
import concourse.tile as tile  # noqa: E402
from concourse import bacc, bass_utils, mybir  # noqa: E402

VOCAB, D, B, S, E = 50257, 512, 8, 2048, 5
P = 128                     # partitions / tokens per tile
NT = S // P                 # 16 token tiles per core
NK = D // P                 # 4 contraction chunks
CHUNKS_PER_EXPERT = [1, 2, 3, 4, 4]   # tail-chunk trick
NJ = sum(CHUNKS_PER_EXPERT)           # 14 (expert, chunk) pairs
# (expert, chunk) pairs in order, packed into ceil(14/4)=4 transpose banks
EK_PAIRS = [(e, k) for e in range(E) for k in range(CHUNKS_PER_EXPERT[e])]
# expert-boundary thresholds in g = gate*512 space: expert e <=> g in (LO[e], HI[e]]
BOUND_LO = [-1.0, 101.0, 203.0, 305.0, 407.0]
BOUND_HI = [101.0, 203.0, 305.0, 407.0, 1e30]

F32 = mybir.dt.float32
F32R = mybir.dt.float32r
I32 = mybir.dt.int32
DA = 528  # augmented row: 512 emb + gate at col 512 + pad to 64B multiple
DG = 512  # gate column within the augmented row


def build_program(debug_taps=False):
    """Build the single-core Tile program (same program runs SPMD on 8 cores)."""
    nc = bacc.Bacc(
        "TRN2",
        target_bir_lowering=False,
        debug=False,
        enable_asserts=False,
        num_devices=8,
    )

    ids = nc.dram_tensor("ids", [P, NT], I32, kind="ExternalInput").ap()
    # augmented table: [:, :512] = emb_table, [:, 512] = gate_table (pad to 528
    # to keep rows 64B-aligned for the gather)
    emb = nc.dram_tensor("emb", [VOCAB, DA], F32, kind="ExternalInput").ap()
    wt = nc.dram_tensor("wt", [P, NJ, D], F32R, kind="ExternalInput").ap()
    biasg = nc.dram_tensor("bias", [E, D], F32, kind="ExternalInput").ap()
    iota = nc.dram_tensor("iota", [P, D], F32, kind="ExternalInput").ap()
    ident = nc.dram_tensor("ident", [P, P], F32R, kind="ExternalInput").ap()
    ident32 = nc.dram_tensor("ident32", [P, P], F32, kind="ExternalInput").ap()
    y = nc.dram_tensor("y", [S, D], F32, kind="ExternalOutput").ap()
    if debug_taps:
        dbg_emb = nc.dram_tensor("dbg_emb", [P, NT, DA], F32, kind="ExternalOutput").ap()
        dbg_gsc = nc.dram_tensor("dbg_gsc", [P, NT], F32, kind="ExternalOutput").ap()
        dbg_ind = nc.dram_tensor("dbg_ind", [P, NT, E], F32, kind="ExternalOutput").ap()
        dbg_xm = nc.dram_tensor("dbg_xm", [P, D], F32, kind="ExternalOutput").ap()
        dbg_xmt = nc.dram_tensor("dbg_xmt", [P, P], F32R, kind="ExternalOutput").ap()

    with tile.TileContext(nc) as tc:
        with (
            tc.tile_pool(name="singles", bufs=1) as singles,
            tc.tile_pool(name="work", bufs=4) as work,
            tc.tile_pool(name="xmt", bufs=3) as xmt,
            tc.tile_pool(name="tp_ps", bufs=2, space="PSUM") as tp_ps,
            tc.tile_pool(name="y_ps", bufs=1, space="PSUM") as y_ps,
        ):
            # ---- constants / global loads ----
            iota_sb = singles.tile([P, D], F32)
            nc.sync.dma_start(out=iota_sb[:], in_=iota[:, :])
            ident_sb = singles.tile([P, P], F32R)
            nc.sync.dma_start(out=ident_sb[:], in_=ident[:, :])
            ident_sb32 = singles.tile([P, P], F32)
            nc.sync.dma_start(out=ident_sb32[:], in_=ident32[:, :])
            wt_sb = singles.tile([P, NJ, D], F32R)
            nc.sync.dma_start(out=wt_sb[:], in_=wt[:, :, :])
            ids_sb = singles.tile([P, NT], I32)
            nc.sync.dma_start(out=ids_sb[:], in_=ids[:, :])

            # gather emb+gate rows per 128-token tile (HW indirect DMA only
            # supports one index per partition): emb_sb[p, t, :] = emb[ids[p,t], :]
            emb_sb = singles.tile([P, NT, DA], F32)
            for t in range(NT):
                nc.gpsimd.indirect_dma_start(
                    out=emb_sb[:, t, :],
                    out_offset=None,
                    in_=emb[:, :],
                    in_offset=bass.IndirectOffsetOnAxis(
                        ap=ids_sb[:, t : t + 1], axis=0
                    ),
                )

            # g = gate * 512 (gate rides at column 512 of the augmented rows)
            gsc = singles.tile([P, NT], F32)
            nc.vector.tensor_scalar(
                out=gsc[:], in0=emb_sb[:, :, DG], scalar1=float(D), scalar2=None,
                op0=mybir.AluOpType.mult,
            )

            # expert indicators ind[p, t, e] = (g in (LO[e], HI[e]])
            ind = singles.tile([P, NT, E], F32)
            tmp = singles.tile([P, NT], F32)
            nc.vector.tensor_scalar(
                out=ind[:, :, 0], in0=gsc[:], scalar1=BOUND_HI[0], scalar2=None,
                op0=mybir.AluOpType.is_le,
            )
            for e in (1, 2, 3):
                nc.vector.tensor_scalar(
                    out=ind[:, :, e], in0=gsc[:], scalar1=BOUND_LO[e], scalar2=None,
                    op0=mybir.AluOpType.is_gt,
                )
                nc.vector.tensor_scalar(
                    out=tmp[:], in0=gsc[:], scalar1=BOUND_HI[e], scalar2=None,
                    op0=mybir.AluOpType.is_le,
                )
                nc.vector.tensor_tensor(
                    out=ind[:, :, e], in0=ind[:, :, e], in1=tmp[:],
                    op=mybir.AluOpType.mult,
                )
            nc.vector.tensor_scalar(
                out=ind[:, :, 4], in0=gsc[:], scalar1=BOUND_LO[4], scalar2=None,
                op0=mybir.AluOpType.is_gt,
            )

            # eidx as int32 for the per-tile bias gathers:
            # eidx = ind1 + 2*ind2 + 3*ind3 + 4*ind4
            eidx_f = singles.tile([P, NT], F32)
            nc.vector.tensor_scalar(
                out=eidx_f[:], in0=ind[:, :, 1], scalar1=1.0, scalar2=None,
                op0=mybir.AluOpType.mult,
            )
            for e in (2, 3, 4):
                nc.vector.tensor_scalar(
                    out=tmp[:], in0=ind[:, :, e], scalar1=float(e), scalar2=None,
                    op0=mybir.AluOpType.mult,
                )
                nc.vector.tensor_tensor(
                    out=eidx_f[:], in0=eidx_f[:], in1=tmp[:],
                    op=mybir.AluOpType.add,
                )
            eidx_i = singles.tile([P, NT], I32)
            nc.vector.tensor_copy(out=eidx_i[:], in_=eidx_f[:])
            ind_i8 = singles.tile([P, NT, E], mybir.dt.int8)
            nc.vector.tensor_copy(out=ind_i8[:], in_=ind[:])
            # per-tile bias rows: bsel[p, t, :] = expert_b[eidx[p, t], :]
            bsel = singles.tile([P, NT, D], F32)
            for t in range(NT):
                nc.gpsimd.indirect_dma_start(
                    out=bsel[:, t, :],
                    out_offset=None,
                    in_=biasg[:, :],
                    in_offset=bass.IndirectOffsetOnAxis(
                        ap=eidx_i[:, t : t + 1], axis=0
                    ),
                )

            if debug_taps:
                nc.sync.dma_start(out=dbg_emb[:, :, :], in_=emb_sb[:])
                nc.sync.dma_start(out=dbg_gsc[:, :], in_=gsc[:])
                nc.sync.dma_start(out=dbg_ind[:, :, :], in_=ind[:])

            # ---- per 128-token tile ----
            for t in range(NT):
                mask = work.tile([P, D], F32, tag="mask")
                nc.vector.tensor_scalar(
                    out=mask[:], in0=iota_sb[:], scalar1=gsc[:, t : t + 1],
                    scalar2=None, op0=mybir.AluOpType.is_lt,
                )
                xm = work.tile([P, D], F32R, tag="xm")
                nc.vector.tensor_tensor(
                    out=xm[:], in0=mask[:], in1=emb_sb[:, t, :D],
                    op=mybir.AluOpType.mult,
                )
                if debug_taps and t == 0:
                    nc.sync.dma_start(out=dbg_xm[:, :], in_=xm[:])

                # transpose the 4 K-chunks of xm into one PSUM bank, then one
                # PSUM->SBUF cast
                tp = tp_ps.tile([P, 4 * P], F32R, tag="tp")
                for k in range(NK):
                    nc.tensor.matmul(
                        out=tp[:, k * P : (k + 1) * P],
                        lhsT=xm[:, k * P : (k + 1) * P],
                        rhs=ident_sb[:],
                        is_transpose=True,
                        start=(k == 0), stop=(k == NK - 1),
                    )
                xT = xmt.tile([P, 4 * P], F32R, tag="xmT")
                if t % 2 == 0:
                    nc.vector.tensor_copy(out=xT[:], in_=tp[:])
                else:
                    nc.scalar.activation(
                        out=xT[:], in_=tp[:],
                        func=mybir.ActivationFunctionType.Copy,
                    )
                if debug_taps and t == 0:
                    nc.sync.dma_start(out=dbg_xmt[:, :], in_=xT[:, :P])

                # one PSUM bank per expert; expert e needs chunks 0..ce-1.
                # k-outer order keeps the same lhsT on consecutive matmuls.
                banks = []
                for e in range(E):
                    ybank = y_ps.tile([P, D], F32, tag=f"yps{e}")
                    banks.append(ybank)
                for k in range(NK):
                    for e in range(E):
                        ce = CHUNKS_PER_EXPERT[e]
                        if k >= ce:
                            continue
                        j = sum(CHUNKS_PER_EXPERT[:e]) + k
                        nc.tensor.matmul(
                            out=banks[e][:],
                            lhsT=xT[:, k * P : (k + 1) * P],
                            rhs=wt_sb[:, j, :],
                            start=(k == 0), stop=(k == ce - 1),
                        )
                # assemble: rows of expert e come from bank e; add bias[eidx]
                y_sb = work.tile([P, D], F32, tag="ysb")
                nc.scalar.activation(
                    out=y_sb[:], in_=banks[0][:],
                    func=mybir.ActivationFunctionType.Copy,
                    scale=ind[:, t, 0 : 1],
                )
                for e in range(1, E):
                    nc.vector.copy_predicated(
                        out=y_sb[:],
                        mask=ind_i8[:, t, e : e + 1].to_broadcast([P, D]),
                        data=banks[e][:],
                    )
                nc.vector.tensor_tensor(
                    out=y_sb[:], in0=y_sb[:], in1=bsel[:, t, :],
                    op=mybir.AluOpType.add,
                )
                nc.sync.dma_start(out=y[t * P : (t + 1) * P, :], in_=y_sb[:])

    nc.compile()
    return nc


def prep_core_inputs(input_ids_row, emb_table, gate_table, expert_w, expert_b,
                     aug=None):
    """Host-side layout prep for one core. input_ids_row: (S,) int."""
    ids = np.ascontiguousarray(
        input_ids_row.reshape(NT, P).T.astype(np.int32)
    )  # [P, NT]: ids[p, t] = token t*128+p
    if aug is None:
        aug = np.zeros((VOCAB, DA), dtype=np.float32)
        aug[:, :D] = emb_table
        aug[:, DG] = gate_table[:, 0]
    # wt[p, j, :] = expert_w[e].T[128k+p, :] = expert_w[e][:, 128k+p] for j=(e,k)
    wt_full = np.transpose(expert_w, (2, 0, 1)).reshape(NK, P, E, D)  # [k,p,e,o]
    cols = []
    for e, k in EK_PAIRS:
        cols.append(wt_full[k, :, e, :])  # [P, D]
    wt = np.ascontiguousarray(np.stack(cols, axis=1), dtype=np.float32)  # [P,NJ,D]
    iota = np.ascontiguousarray(
        np.broadcast_to(np.arange(D, dtype=np.float32), (P, D))
    )
    ident = np.eye(P, dtype=np.float32)
    return {
        "ids": ids,
        "emb": aug,
        "wt": wt,
        "bias": np.ascontiguousarray(expert_b, dtype=np.float32),
        "iota": iota,
        "ident": ident,
        "ident32": ident,
    }


_CACHED_NC = None


def kernel(input_ids, emb_table, gate_table, expert_w, expert_b):
    global _CACHED_NC
    input_ids = np.asarray(input_ids)
    emb_table = np.asarray(emb_table, dtype=np.float32)
    gate_table = np.asarray(gate_table, dtype=np.float32)
    expert_w = np.asarray(expert_w, dtype=np.float32)
    expert_b = np.asarray(expert_b, dtype=np.float32)

    if _CACHED_NC is None:
        _CACHED_NC = build_program()
    nc = _CACHED_NC

    shared = None
    in_maps = []
    for c in range(B):
        m = prep_core_inputs(
            input_ids[c], emb_table, gate_table, expert_w, expert_b,
            aug=None if shared is None else shared["emb"],
        )
        if shared is None:
            shared = m
        else:
            # reuse identical replicated arrays across cores
            for k_ in ("emb", "wt", "bias", "iota", "ident", "ident32"):
                m[k_] = shared[k_]
        in_maps.append(m)

    trace = bool(int(os.environ.get("BASS_KERNEL_TRACE", "0")))
    res = bass_utils.run_bass_kernel_spmd(
        nc, in_maps, core_ids=list(range(B)), trace=trace
    )
    kernel.last_result = res
    out = np.stack([res.results[c]["y"] for c in range(B)], axis=0)
    return out.astype(np.float32)


# revision 33
# speedup vs baseline: 1.5187x; 1.0967x over previous
"""Trainium2 Bass kernel for nn_DifferentiableEmbedding (moe_routing).

Computation (per token t):
    data = emb_table[id]                      # (512,)
    g    = gate_table[id] * 512               # scalar in (0.512, 512)
    mask = (iota512 < g)                      # 0/1 mask (frac term is exactly 0 in f32)
    e    = clip(ceil(g) // 102, 0, 4)         # expert index
    y    = (data*mask) @ W[e].T + b[e]

Sharding: data-parallel on B (8 batch rows -> 8 cores). Tables and expert
weights replicated per core.

Key design points:
  * count = sum(mask) = ceil(g) exactly in f32 (the straight-through frac term
    rounds to exactly 0), so the expert index and selected bias row are pure
    functions of the vocab id.  e(v) and expert_b[e(v)] are therefore
    precomputed on the host from gate_table/expert_b (weights-only prep) and
    appended to each embedding-table row; one indirect gather per 128-token
    tile fetches [emb | gate | e(v) | bias-row] together.  (HW indirect DMA
    honors only one index per partition, so gathers are per-tile.)
  * tokens of expert e have mask zero beyond feature 102e+101, so expert e
    only needs the first ceil((102e+101)/128) of the 4 K-chunks: [1,2,3,4,4]
    -> 14 accumulating matmuls per 128-token tile instead of 20.
  * xm is transposed once per tile (4 PE transposes into one PSUM bank, one
    PSUM->SBUF cast); the 14 matmuls write 5 per-expert PSUM banks and the
    output rows are assembled with one ACT scale-copy + 4 predicated copies
    selected by the expert indicators, plus the gathered bias row.
  * matmuls run as float32r (full PE rate at N=512).
"""

import os
import sys

import numpy as np

sys.path.insert(0, "/opt/trn_rl_repo")

import concourse.bass as bass  # noqa: E402
import concourse.tile as tile  # noqa: E402
from concourse import bacc, bass_utils, mybir  # noqa: E402

VOCAB, D, B, S, E = 50257, 512, 8, 2048, 5
P = 128                     # partitions / tokens per tile
NT = S // P                 # 16 token tiles per core
NK = D // P                 # 4 contraction chunks
CHUNKS_PER_EXPERT = [1, 2, 3, 4, 4]   # tail-chunk trick
NJ = sum(CHUNKS_PER_EXPERT)           # 14 (expert, chunk) pairs

F32 = mybir.dt.float32
F32R = mybir.dt.float32r
I32 = mybir.dt.int32
I8 = mybir.dt.int8
# augmented row: [0:512] emb, [512] gate, [513] e(v), [514:528] pad,
# [528:1040] bias row of e(v)  -> 1040 f32 = 4160 B (64B-aligned)
DA = 1040
DG = 512   # gate column
DE = 513   # expert-index column
DB = 528   # bias row start
NH = NT // 2  # tiles per indicator half


def build_program(debug_taps=False):
    """Build the single-core Tile program (same program runs SPMD on 8 cores)."""
    nc = bacc.Bacc(
        "TRN2",
        target_bir_lowering=False,
        debug=False,
        enable_asserts=False,
        num_devices=8,
    )

    ids = nc.dram_tensor("ids", [P, NT], I32, kind="ExternalInput").ap()
    emb = nc.dram_tensor("emb", [VOCAB, DA], F32, kind="ExternalInput").ap()
    wt = nc.dram_tensor("wt", [P, NJ, D], F32R, kind="ExternalInput").ap()
    iota = nc.dram_tensor("iota", [P, D], F32, kind="ExternalInput").ap()
    ident = nc.dram_tensor("ident", [P, P], F32R, kind="ExternalInput").ap()
    y = nc.dram_tensor("y", [S, D], F32, kind="ExternalOutput").ap()
    if debug_taps:
        dbg_emb = nc.dram_tensor("dbg_emb", [P, NT, DA], F32, kind="ExternalOutput").ap()
        dbg_gsc = nc.dram_tensor("dbg_gsc", [P, NT], F32, kind="ExternalOutput").ap()
        dbg_ind = nc.dram_tensor("dbg_ind", [P, NT, E], F32, kind="ExternalOutput").ap()
        dbg_xm = nc.dram_tensor("dbg_xm", [P, D], F32, kind="ExternalOutput").ap()
        dbg_xmt = nc.dram_tensor("dbg_xmt", [P, P], F32R, kind="ExternalOutput").ap()

    with tile.TileContext(nc) as tc:
        with (
            tc.tile_pool(name="singles", bufs=1) as singles,
            tc.tile_pool(name="work", bufs=4) as work,
            tc.tile_pool(name="xmt", bufs=3) as xmt,
            tc.tile_pool(name="tp_ps", bufs=2, space="PSUM") as tp_ps,
            tc.tile_pool(name="y_ps", bufs=1, space="PSUM") as y_ps,
        ):
            # ids go first, on the scalar-engine HWDGE queue, so the gathers
            # are not stuck behind the big weight DMA on the sync queue
            ids_sb = singles.tile([P, NT], I32)
            nc.scalar.dma_start(out=ids_sb[:], in_=ids[:, :])

            # gather [emb | gate | e(v) | bias] rows per 128-token tile
            emb_sb = singles.tile([P, NT, DA], F32)
            for t in range(NT):
                nc.gpsimd.indirect_dma_start(
                    out=emb_sb[:, t, :],
                    out_offset=None,
                    in_=emb[:, :],
                    in_offset=bass.IndirectOffsetOnAxis(
                        ap=ids_sb[:, t : t + 1], axis=0
                    ),
                )

            # ---- constants (sync queue, overlaps the gathers) ----
            iota_sb = singles.tile([P, D], F32)
            nc.sync.dma_start(out=iota_sb[:], in_=iota[:, :])
            ident_sb = singles.tile([P, P], F32R)
            nc.sync.dma_start(out=ident_sb[:], in_=ident[:, :])
            wt_sb = singles.tile([P, NJ, D], F32R)
            nc.sync.dma_start(out=wt_sb[:], in_=wt[:, :, :])

            # gate*512 and expert indicators, computed per half so the first
            # tiles can start before the later gathers land
            gsc = singles.tile([P, NT], F32)
            ind_i8 = singles.tile([P, NT, E], I8)
            ind_f = singles.tile([P, NT, E], F32)
            for h in range(2):
                hs = slice(h * NH, (h + 1) * NH)
                nc.vector.tensor_scalar(
                    out=gsc[:, hs], in0=emb_sb[:, hs, DG], scalar1=float(D),
                    scalar2=None, op0=mybir.AluOpType.mult,
                )
                for e in range(E):
                    nc.vector.tensor_scalar(
                        out=ind_f[:, hs, e], in0=emb_sb[:, hs, DE],
                        scalar1=float(e), scalar2=None,
                        op0=mybir.AluOpType.is_equal,
                    )
                nc.vector.tensor_copy(out=ind_i8[:, hs, :], in_=ind_f[:, hs, :])

            if debug_taps:
                nc.sync.dma_start(out=dbg_emb[:, :, :], in_=emb_sb[:])
                nc.sync.dma_start(out=dbg_gsc[:, :], in_=gsc[:])
                nc.sync.dma_start(out=dbg_ind[:, :, :], in_=ind_f[:])

            # ---- per 128-token tile ----
            for t in range(NT):
                mask = work.tile([P, D], F32, tag="mask")
                nc.vector.tensor_scalar(
                    out=mask[:], in0=iota_sb[:], scalar1=gsc[:, t : t + 1],
                    scalar2=None, op0=mybir.AluOpType.is_lt,
                )
                xm = work.tile([P, D], F32R, tag="xm")
                nc.vector.tensor_tensor(
                    out=xm[:], in0=mask[:], in1=emb_sb[:, t, :D],
                    op=mybir.AluOpType.mult,
                )
                if debug_taps and t == 0:
                    nc.sync.dma_start(out=dbg_xm[:, :], in_=xm[:])

                # transpose the 4 K-chunks of xm into one PSUM bank, then one
                # PSUM->SBUF cast
                tp = tp_ps.tile([P, 4 * P], F32R, tag="tp")
                for k in range(NK):
                    nc.tensor.matmul(
                        out=tp[:, k * P : (k + 1) * P],
                        lhsT=xm[:, k * P : (k + 1) * P],
                        rhs=ident_sb[:],
                        is_transpose=True,
                        start=(k == 0), stop=(k == NK - 1),
                    )
                xT = xmt.tile([P, 4 * P], F32R, tag="xmT")
                if t % 2 == 0:
                    nc.vector.tensor_copy(out=xT[:], in_=tp[:])
                else:
                    nc.scalar.activation(
                        out=xT[:], in_=tp[:],
                        func=mybir.ActivationFunctionType.Copy,
                    )
                if debug_taps and t == 0:
                    nc.sync.dma_start(out=dbg_xmt[:, :], in_=xT[:, :P])

                # one PSUM bank per expert; expert e needs chunks 0..ce-1
                banks = []
                for e in range(E):
                    ybank = y_ps.tile([P, D], F32, tag=f"yps{e}")
                    banks.append(ybank)
                for e in range(E):
                    ce = CHUNKS_PER_EXPERT[e]
                    for k in range(ce):
                        j = sum(CHUNKS_PER_EXPERT[:e]) + k
                        nc.tensor.matmul(
                            out=banks[e][:],
                            lhsT=xT[:, k * P : (k + 1) * P],
                            rhs=wt_sb[:, j, :],
                            start=(k == 0), stop=(k == ce - 1),
                        )
                # assemble: rows of expert e from bank e, then add bias row
                y_sb = work.tile([P, D], F32, tag="ysb")
                nc.scalar.activation(
                    out=y_sb[:], in_=banks[0][:],
                    func=mybir.ActivationFunctionType.Copy,
                    scale=ind_f[:, t, 0 : 1],
                )
                for e in range(1, E):
                    nc.vector.copy_predicated(
                        out=y_sb[:],
                        mask=ind_i8[:, t, e : e + 1].to_broadcast([P, D]),
                        data=banks[e][:],
                    )
                nc.vector.tensor_tensor(
                    out=y_sb[:], in0=y_sb[:], in1=emb_sb[:, t, DB:],
                    op=mybir.AluOpType.add,
                )
                nc.sync.dma_start(out=y[t * P : (t + 1) * P, :], in_=y_sb[:])

    nc.compile()
    return nc


def prep_core_inputs(input_ids_row, emb_table, gate_table, expert_w, expert_b,
                     aug=None):
    """Host-side layout prep for one core. input_ids_row: (S,) int."""
    ids = np.ascontiguousarray(
        input_ids_row.reshape(NT, P).T.astype(np.int32)
    )  # [P, NT]: ids[p, t] = token t*128+p
    if aug is None:
        aug = build_aug_table(emb_table, gate_table, expert_b)
    # wt[p, j, :] = expert_w[e].T[128k+p, :] = expert_w[e][:, 128k+p] for j=(e,k)
    wt_full = np.transpose(expert_w, (2, 0, 1)).reshape(NK, P, E, D)  # [k,p,e,o]
    cols = []
    for e in range(E):
        for k in range(CHUNKS_PER_EXPERT[e]):
            cols.append(wt_full[k, :, e, :])  # [P, D]
    wt = np.ascontiguousarray(np.stack(cols, axis=1), dtype=np.float32)  # [P,NJ,D]
    iota = np.ascontiguousarray(
        np.broadcast_to(np.arange(D, dtype=np.float32), (P, D))
    )
    ident = np.eye(P, dtype=np.float32)
    return {
        "ids": ids,
        "emb": aug,
        "wt": wt,
        "iota": iota,
        "ident": ident,
    }


def build_aug_table(emb_table, gate_table, expert_b):
    """Weights-only preprocessing: per vocab row v append gate, expert index
    e(v) = clip(ceil(gate*512)//102, 0, 4), and the selected bias row."""
    g = gate_table[:, 0].astype(np.float32) * np.float32(D)
    count = np.ceil(g)
    eidx = np.clip((count // float(D // E)).astype(np.int64), 0, E - 1)
    aug = np.zeros((VOCAB, DA), dtype=np.float32)
    aug[:, :D] = emb_table
    aug[:, DG] = gate_table[:, 0]
    aug[:, DE] = eidx.astype(np.float32)
    aug[:, DB:] = expert_b[eidx]
    return aug


_CACHED_NC = None


def kernel(input_ids, emb_table, gate_table, expert_w, expert_b):
    global _CACHED_NC
    input_ids = np.asarray(input_ids)
    emb_table = np.asarray(emb_table, dtype=np.float32)
    gate_table = np.asarray(gate_table, dtype=np.float32)
    expert_w = np.asarray(expert_w, dtype=np.float32)
    expert_b = np.asarray(expert_b, dtype=np.float32)

    if _CACHED_NC is None:
        _CACHED_NC = build_program()
    nc = _CACHED_NC

    shared = None
    in_maps = []
    for c in range(B):
        m = prep_core_inputs(
            input_ids[c], emb_table, gate_table, expert_w, expert_b,
            aug=None if shared is None else shared["emb"],
        )
        if shared is None:
            shared = m
        else:
            # reuse identical replicated arrays across cores
            for k_ in ("emb", "wt", "iota", "ident"):
                m[k_] = shared[k_]
        in_maps.append(m)

    trace = bool(int(os.environ.get("BASS_KERNEL_TRACE", "0")))
    res = bass_utils.run_bass_kernel_spmd(
        nc, in_maps, core_ids=list(range(B)), trace=trace
    )
    kernel.last_result = res
    out = np.stack([res.results[c]["y"] for c in range(B)], axis=0)
    return out.astype(np.float32)


# revision 35
# speedup vs baseline: 1.5970x; 1.0515x over previous
"""Trainium2 Bass kernel for nn_DifferentiableEmbedding (moe_routing).

Computation (per token t):
    data = emb_table[id]                      # (512,)
    g    = gate_table[id] * 512               # scalar in (0.512, 512)
    mask = (iota512 < g)                      # 0/1 mask (frac term is exactly 0 in f32)
    e    = clip(ceil(g) // 102, 0, 4)         # expert index
    y    = (data*mask) @ W[e].T + b[e]

Sharding: data-parallel on B (8 batch rows -> 8 cores). Tables and expert
weights replicated per core.

Key design points:
  * count = sum(mask) = ceil(g) exactly in f32 (the straight-through frac term
    rounds to exactly 0), so the expert index and selected bias row are pure
    functions of the vocab id.  e(v) and expert_b[e(v)] are therefore
    precomputed on the host from gate_table/expert_b (weights-only prep) and
    appended to each embedding-table row; one indirect gather per 128-token
    tile fetches [emb | gate | e(v) | bias-row] together.  (HW indirect DMA
    honors only one index per partition, so gathers are per-tile.)
  * tokens of expert e have mask zero beyond feature 102e+101, so expert e
    only needs the first ceil((102e+101)/128) of the 4 K-chunks: [1,2,3,4,4]
    -> 14 accumulating matmuls per 128-token tile instead of 20.
  * xm is transposed once per tile (4 PE transposes into one PSUM bank, one
    PSUM->SBUF cast); the 14 matmuls write 5 per-expert PSUM banks and the
    output rows are assembled with one ACT scale-copy + 4 predicated copies
    selected by the expert indicators, plus the gathered bias row.
  * matmuls run as float32r (full PE rate at N=512).
"""

import os
import sys

import numpy as np

sys.path.insert(0, "/opt/trn_rl_repo")

import concourse.bass as bass  # noqa: E402
import concourse.tile as tile  # noqa: E402
from concourse import bacc, bass_utils, mybir  # noqa: E402

VOCAB, D, B, S, E = 50257, 512, 8, 2048, 5
P = 128                     # partitions / tokens per tile
NT = S // P                 # 16 token tiles per core
NK = D // P                 # 4 contraction chunks
CHUNKS_PER_EXPERT = [1, 2, 3, 4, 4]   # tail-chunk trick
NJ = sum(CHUNKS_PER_EXPERT)           # 14 (expert, chunk) pairs

F32 = mybir.dt.float32
F32R = mybir.dt.float32r
I32 = mybir.dt.int32
I8 = mybir.dt.int8
# augmented row: [0:512] emb, [512] gate, [513] e(v), [514:528] pad,
# [528:1040] bias row of e(v)  -> 1040 f32 = 4160 B (64B-aligned)
DA = 1040
DG = 512   # gate column
DE = 513   # expert-index column
DB = 528   # bias row start
NH = NT // 2  # tiles per indicator half


def build_program(debug_taps=False):
    """Build the single-core Tile program (same program runs SPMD on 8 cores)."""
    nc = bacc.Bacc(
        "TRN2",
        target_bir_lowering=False,
        debug=False,
        enable_asserts=False,
        num_devices=8,
    )

    ids = nc.dram_tensor("ids", [P, NT], I32, kind="ExternalInput").ap()
    emb = nc.dram_tensor("emb", [VOCAB, DA], F32, kind="ExternalInput").ap()
    wt = nc.dram_tensor("wt", [P, NJ, D], F32R, kind="ExternalInput").ap()
    iota = nc.dram_tensor("iota", [P, D], F32, kind="ExternalInput").ap()
    ident = nc.dram_tensor("ident", [P, P], F32R, kind="ExternalInput").ap()
    iota5 = nc.dram_tensor("iota5", [P, E], F32, kind="ExternalInput").ap()
    y = nc.dram_tensor("y", [S, D], F32, kind="ExternalOutput").ap()
    if debug_taps:
        dbg_emb = nc.dram_tensor("dbg_emb", [P, NT, DA], F32, kind="ExternalOutput").ap()
        dbg_gsc = nc.dram_tensor("dbg_gsc", [P, NT], F32, kind="ExternalOutput").ap()
        dbg_ind = nc.dram_tensor("dbg_ind", [P, NT, E], F32, kind="ExternalOutput").ap()
        dbg_xm = nc.dram_tensor("dbg_xm", [P, D], F32, kind="ExternalOutput").ap()
        dbg_xmt = nc.dram_tensor("dbg_xmt", [P, P], F32R, kind="ExternalOutput").ap()

    with tile.TileContext(nc) as tc:
        with (
            tc.tile_pool(name="singles", bufs=1) as singles,
            tc.tile_pool(name="work", bufs=4) as work,
            tc.tile_pool(name="xmt", bufs=12) as xmt,
            tc.tile_pool(name="gpool", bufs=1) as gpool,
            tc.tile_pool(name="tp_ps", bufs=2, space="PSUM") as tp_ps,
            tc.tile_pool(name="y_ps", bufs=1, space="PSUM") as y_ps,
        ):
            # ids go first, on the scalar-engine HWDGE queue, so the gathers
            # are not stuck behind the big weight DMA on the sync queue
            ids_sb = singles.tile([P, NT], I32)
            nc.scalar.dma_start(out=ids_sb[:], in_=ids[:, :])

            # ---- constants (sync queue, overlaps the gathers) ----
            iota_sb = singles.tile([P, D], F32)
            nc.sync.dma_start(out=iota_sb[:], in_=iota[:, :])
            ident_sb = singles.tile([P, P], F32R)
            nc.sync.dma_start(out=ident_sb[:], in_=ident[:, :])
            iota5_sb = singles.tile([P, E], F32)
            nc.sync.dma_start(out=iota5_sb[:], in_=iota5[:, :])
            wt_sb = singles.tile([P, NJ, D], F32R)
            nc.sync.dma_start(out=wt_sb[:], in_=wt[:, :, :])

            # gather [emb | gate | e(v) | bias] rows per 128-token tile; one
            # SBUF tile per gather so downstream deps are exact
            embs = []
            for t in range(NT):
                emb_t = gpool.tile([P, DA], F32, tag=f"emb{t}")
                nc.gpsimd.indirect_dma_start(
                    out=emb_t[:],
                    out_offset=None,
                    in_=emb[:, :],
                    in_offset=bass.IndirectOffsetOnAxis(
                        ap=ids_sb[:, t : t + 1], axis=0
                    ),
                )
                embs.append(emb_t)

            if debug_taps:
                for t in range(NT):
                    nc.sync.dma_start(out=dbg_emb[:, t, :], in_=embs[t][:])

            # ---- per 128-token tile ----
            for t in range(NT):
                emb_t = embs[t]
                # g = gate*512 (must round exactly like the reference)
                gsc_t = work.tile([P, 1], F32, tag="gsc")
                nc.vector.tensor_scalar(
                    out=gsc_t[:], in0=emb_t[:, DG : DG + 1], scalar1=float(D),
                    scalar2=None, op0=mybir.AluOpType.mult,
                )
                # one-hot expert indicators from the precomputed e(v) column
                ind_f = work.tile([P, E], F32, tag="indf")
                nc.vector.tensor_scalar(
                    out=ind_f[:], in0=iota5_sb[:], scalar1=emb_t[:, DE : DE + 1],
                    scalar2=None, op0=mybir.AluOpType.is_equal,
                )
                ind_i8 = work.tile([P, E], I8, tag="indi")
                nc.vector.tensor_copy(out=ind_i8[:], in_=ind_f[:])
                if debug_taps:
                    nc.sync.dma_start(out=dbg_gsc[:, t : t + 1], in_=gsc_t[:])
                    nc.sync.dma_start(out=dbg_ind[:, t, :], in_=ind_f[:])

                mask = work.tile([P, D], F32, tag="mask")
                nc.vector.tensor_scalar(
                    out=mask[:], in0=iota_sb[:], scalar1=gsc_t[:],
                    scalar2=None, op0=mybir.AluOpType.is_lt,
                )
                xm = work.tile([P, D], F32R, tag="xm")
                nc.vector.tensor_tensor(
                    out=xm[:], in0=mask[:], in1=emb_t[:, :D],
                    op=mybir.AluOpType.mult,
                )
                if debug_taps and t == 0:
                    nc.sync.dma_start(out=dbg_xm[:, :], in_=xm[:])

                # transpose the 4 K-chunks of xm into one PSUM bank; separate
                # xT tiles per chunk (cast split across DVE/ACT)
                tp = tp_ps.tile([P, 4 * P], F32R, tag="tp")
                for k in range(NK):
                    nc.tensor.matmul(
                        out=tp[:, k * P : (k + 1) * P],
                        lhsT=xm[:, k * P : (k + 1) * P],
                        rhs=ident_sb[:],
                        is_transpose=True,
                        start=(k == 0), stop=(k == NK - 1),
                    )
                xTs = []
                for k in range(NK):
                    xT_k = xmt.tile([P, P], F32R, tag=f"xmT{k}")
                    if k % 2 == 0:
                        nc.vector.tensor_copy(
                            out=xT_k[:], in_=tp[:, k * P : (k + 1) * P]
                        )
                    else:
                        nc.scalar.activation(
                            out=xT_k[:], in_=tp[:, k * P : (k + 1) * P],
                            func=mybir.ActivationFunctionType.Copy,
                        )
                    xTs.append(xT_k)
                if debug_taps and t == 0:
                    nc.sync.dma_start(out=dbg_xmt[:, :], in_=xTs[0][:])

                # one PSUM bank per expert; expert e needs chunks 0..ce-1
                banks = []
                for e in range(E):
                    ybank = y_ps.tile([P, D], F32, tag=f"yps{e}")
                    banks.append(ybank)
                for e in range(E):
                    ce = CHUNKS_PER_EXPERT[e]
                    for k in range(ce):
                        j = sum(CHUNKS_PER_EXPERT[:e]) + k
                        nc.tensor.matmul(
                            out=banks[e][:],
                            lhsT=xTs[k][:],
                            rhs=wt_sb[:, j, :],
                            start=(k == 0), stop=(k == ce - 1),
                        )
                # assemble: rows of expert e from bank e, then add bias row
                y_sb = work.tile([P, D], F32, tag="ysb")
                nc.scalar.activation(
                    out=y_sb[:], in_=banks[0][:],
                    func=mybir.ActivationFunctionType.Copy,
                    scale=ind_f[:, 0:1],
                )
                for e in range(1, E):
                    nc.vector.copy_predicated(
                        out=y_sb[:],
                        mask=ind_i8[:, e : e + 1].to_broadcast([P, D]),
                        data=banks[e][:],
                    )
                nc.vector.tensor_tensor(
                    out=y_sb[:], in0=y_sb[:], in1=emb_t[:, DB:],
                    op=mybir.AluOpType.add,
                )
                nc.sync.dma_start(out=y[t * P : (t + 1) * P, :], in_=y_sb[:])

    nc.compile()
    return nc


def prep_core_inputs(input_ids_row, emb_table, gate_table, expert_w, expert_b,
                     aug=None):
    """Host-side layout prep for one core. input_ids_row: (S,) int."""
    ids = np.ascontiguousarray(
        input_ids_row.reshape(NT, P).T.astype(np.int32)
    )  # [P, NT]: ids[p, t] = token t*128+p
    if aug is None:
        aug = build_aug_table(emb_table, gate_table, expert_b)
    # wt[p, j, :] = expert_w[e].T[128k+p, :] = expert_w[e][:, 128k+p] for j=(e,k)
    wt_full = np.transpose(expert_w, (2, 0, 1)).reshape(NK, P, E, D)  # [k,p,e,o]
    cols = []
    for e in range(E):
        for k in range(CHUNKS_PER_EXPERT[e]):
            cols.append(wt_full[k, :, e, :])  # [P, D]
    wt = np.ascontiguousarray(np.stack(cols, axis=1), dtype=np.float32)  # [P,NJ,D]
    iota = np.ascontiguousarray(
        np.broadcast_to(np.arange(D, dtype=np.float32), (P, D))
    )
    ident = np.eye(P, dtype=np.float32)
    iota5 = np.ascontiguousarray(
        np.broadcast_to(np.arange(E, dtype=np.float32), (P, E))
    )
    return {
        "ids": ids,
        "emb": aug,
        "wt": wt,
        "iota": iota,
        "ident": ident,
        "iota5": iota5,
    }


def build_aug_table(emb_table, gate_table, expert_b):
    """Weights-only preprocessing: per vocab row v append gate, expert index
    e(v) = clip(ceil(gate*512)//102, 0, 4), and the selected bias row."""
    g = gate_table[:, 0].astype(np.float32) * np.float32(D)
    count = np.ceil(g)
    eidx = np.clip((count // float(D // E)).astype(np.int64), 0, E - 1)
    aug = np.zeros((VOCAB, DA), dtype=np.float32)
    aug[:, :D] = emb_table
    aug[:, DG] = gate_table[:, 0]
    aug[:, DE] = eidx.astype(np.float32)
    aug[:, DB:] = expert_b[eidx]
    return aug


_CACHED_NC = None


def kernel(input_ids, emb_table, gate_table, expert_w, expert_b):
    global _CACHED_NC
    input_ids = np.asarray(input_ids)
    emb_table = np.asarray(emb_table, dtype=np.float32)
    gate_table = np.asarray(gate_table, dtype=np.float32)
    expert_w = np.asarray(expert_w, dtype=np.float32)
    expert_b = np.asarray(expert_b, dtype=np.float32)

    if _CACHED_NC is None:
        _CACHED_NC = build_program()
    nc = _CACHED_NC

    shared = None
    in_maps = []
    for c in range(B):
        m = prep_core_inputs(
            input_ids[c], emb_table, gate_table, expert_w, expert_b,
            aug=None if shared is None else shared["emb"],
        )
        if shared is None:
            shared = m
        else:
            # reuse identical replicated arrays across cores
            for k_ in ("emb", "wt", "iota", "ident", "iota5"):
                m[k_] = shared[k_]
        in_maps.append(m)

    trace = bool(int(os.environ.get("BASS_KERNEL_TRACE", "0")))
    res = bass_utils.run_bass_kernel_spmd(
        nc, in_maps, core_ids=list(range(B)), trace=trace
    )
    kernel.last_result = res
    out = np.stack([res.results[c]["y"] for c in range(B)], axis=0)
    return out.astype(np.float32)


# revision 36
# speedup vs baseline: 1.6004x; 1.0021x over previous
"""Trainium2 Bass kernel for nn_DifferentiableEmbedding (moe_routing).

Computation (per token t):
    data = emb_table[id]                      # (512,)
    g    = gate_table[id] * 512               # scalar in (0.512, 512)
    mask = (iota512 < g)                      # 0/1 mask (frac term is exactly 0 in f32)
    e    = clip(ceil(g) // 102, 0, 4)         # expert index
    y    = (data*mask) @ W[e].T + b[e]

Sharding: data-parallel on B (8 batch rows -> 8 cores). Tables and expert
weights replicated per core.

Key design points:
  * count = sum(mask) = ceil(g) exactly in f32 (the straight-through frac term
    rounds to exactly 0), so the expert index and selected bias row are pure
    functions of the vocab id.  e(v) and expert_b[e(v)] are therefore
    precomputed on the host from gate_table/expert_b (weights-only prep) and
    appended to each embedding-table row; one indirect gather per 128-token
    tile fetches [emb | gate | e(v) | bias-row] together.  (HW indirect DMA
    honors only one index per partition, so gathers are per-tile.)
  * tokens of expert e have mask zero beyond feature 102e+101, so expert e
    only needs the first ceil((102e+101)/128) of the 4 K-chunks: [1,2,3,4,4]
    -> 14 accumulating matmuls per 128-token tile instead of 20.
  * xm is transposed once per tile (4 PE transposes into one PSUM bank, one
    PSUM->SBUF cast); the 14 matmuls write 5 per-expert PSUM banks and the
    output rows are assembled with one ACT scale-copy + 4 predicated copies
    selected by the expert indicators, plus the gathered bias row.
  * matmuls run as float32r (full PE rate at N=512).
"""

import os
import sys

import numpy as np

sys.path.insert(0, "/opt/trn_rl_repo")

import concourse.bass as bass  # noqa: E402
import concourse.tile as tile  # noqa: E402
from concourse import bacc, bass_utils, mybir  # noqa: E402

VOCAB, D, B, S, E = 50257, 512, 8, 2048, 5
P = 128                     # partitions / tokens per tile
NT = S // P                 # 16 token tiles per core
NK = D // P                 # 4 contraction chunks
CHUNKS_PER_EXPERT = [1, 2, 3, 4, 4]   # tail-chunk trick
NJ = sum(CHUNKS_PER_EXPERT)           # 14 (expert, chunk) pairs

F32 = mybir.dt.float32
F32R = mybir.dt.float32r
I32 = mybir.dt.int32
I8 = mybir.dt.int8
# augmented row: [0:512] emb, [512] gate, [513] e(v), [514:528] pad,
# [528:1040] bias row of e(v)  -> 1040 f32 = 4160 B (64B-aligned)
DA = 1040
DG = 512   # gate column
DE = 513   # expert-index column
DB = 528   # bias row start
NH = NT // 2  # tiles per indicator half


def build_program(debug_taps=False):
    """Build the single-core Tile program (same program runs SPMD on 8 cores)."""
    nc = bacc.Bacc(
        "TRN2",
        target_bir_lowering=False,
        debug=False,
        enable_asserts=False,
        num_devices=8,
    )

    ids = nc.dram_tensor("ids", [P, NT], I32, kind="ExternalInput").ap()
    emb = nc.dram_tensor("emb", [VOCAB, DA], F32, kind="ExternalInput").ap()
    wt = nc.dram_tensor("wt", [P, NJ, D], F32R, kind="ExternalInput").ap()
    iota = nc.dram_tensor("iota", [P, D], F32, kind="ExternalInput").ap()
    ident = nc.dram_tensor("ident", [P, P], F32R, kind="ExternalInput").ap()
    iota5 = nc.dram_tensor("iota5", [P, E], F32, kind="ExternalInput").ap()
    y = nc.dram_tensor("y", [S, D], F32, kind="ExternalOutput").ap()
    if debug_taps:
        dbg_emb = nc.dram_tensor("dbg_emb", [P, NT, DA], F32, kind="ExternalOutput").ap()
        dbg_gsc = nc.dram_tensor("dbg_gsc", [P, NT], F32, kind="ExternalOutput").ap()
        dbg_ind = nc.dram_tensor("dbg_ind", [P, NT, E], F32, kind="ExternalOutput").ap()
        dbg_xm = nc.dram_tensor("dbg_xm", [P, D], F32, kind="ExternalOutput").ap()
        dbg_xmt = nc.dram_tensor("dbg_xmt", [P, P], F32R, kind="ExternalOutput").ap()

    with tile.TileContext(nc) as tc:
        with (
            tc.tile_pool(name="singles", bufs=1) as singles,
            tc.tile_pool(name="work", bufs=4) as work,
            tc.tile_pool(name="xmt", bufs=12) as xmt,
            tc.tile_pool(name="gpool", bufs=1) as gpool,
            tc.tile_pool(name="tp_ps", bufs=2, space="PSUM") as tp_ps,
            tc.tile_pool(name="y_ps", bufs=1, space="PSUM") as y_ps,
        ):
            # ids go first, on the scalar-engine HWDGE queue, so the gathers
            # are not stuck behind the big weight DMA on the sync queue
            ids_sb = singles.tile([P, NT], I32)
            nc.scalar.dma_start(out=ids_sb[:], in_=ids[:, :])

            # ---- constants (sync queue, overlaps the gathers) ----
            iota_sb = singles.tile([P, D], F32)
            nc.sync.dma_start(out=iota_sb[:], in_=iota[:, :])
            ident_sb = singles.tile([P, P], F32R)
            nc.sync.dma_start(out=ident_sb[:], in_=ident[:, :])
            iota5_sb = singles.tile([P, E], F32)
            nc.sync.dma_start(out=iota5_sb[:], in_=iota5[:, :])
            wt_sb = singles.tile([P, NJ, D], F32R)
            for j in range(NJ):
                nc.sync.dma_start(out=wt_sb[:, j, :], in_=wt[:, j, :])

            # gather [emb | gate | e(v) | bias] rows per 128-token tile; one
            # SBUF tile per gather so downstream deps are exact
            embs = []
            for t in range(NT):
                emb_t = gpool.tile([P, DA], F32, tag=f"emb{t}")
                nc.gpsimd.indirect_dma_start(
                    out=emb_t[:],
                    out_offset=None,
                    in_=emb[:, :],
                    in_offset=bass.IndirectOffsetOnAxis(
                        ap=ids_sb[:, t : t + 1], axis=0
                    ),
                )
                embs.append(emb_t)

            if debug_taps:
                for t in range(NT):
                    nc.sync.dma_start(out=dbg_emb[:, t, :], in_=embs[t][:])

            # ---- per 128-token tile ----
            for t in range(NT):
                emb_t = embs[t]
                # g = gate*512 (must round exactly like the reference)
                gsc_t = work.tile([P, 1], F32, tag="gsc")
                nc.vector.tensor_scalar(
                    out=gsc_t[:], in0=emb_t[:, DG : DG + 1], scalar1=float(D),
                    scalar2=None, op0=mybir.AluOpType.mult,
                )
                # one-hot expert indicators from the precomputed e(v) column
                ind_f = work.tile([P, E], F32, tag="indf")
                nc.vector.tensor_scalar(
                    out=ind_f[:], in0=iota5_sb[:], scalar1=emb_t[:, DE : DE + 1],
                    scalar2=None, op0=mybir.AluOpType.is_equal,
                )
                ind_i8 = work.tile([P, E], I8, tag="indi")
                nc.scalar.activation(
                    out=ind_i8[:], in_=ind_f[:],
                    func=mybir.ActivationFunctionType.Copy,
                )
                if debug_taps:
                    nc.sync.dma_start(out=dbg_gsc[:, t : t + 1], in_=gsc_t[:])
                    nc.sync.dma_start(out=dbg_ind[:, t, :], in_=ind_f[:])

                mask = work.tile([P, D], F32, tag="mask")
                nc.vector.tensor_scalar(
                    out=mask[:], in0=iota_sb[:], scalar1=gsc_t[:],
                    scalar2=None, op0=mybir.AluOpType.is_lt,
                )
                xm = work.tile([P, D], F32R, tag="xm")
                nc.vector.tensor_tensor(
                    out=xm[:], in0=mask[:], in1=emb_t[:, :D],
                    op=mybir.AluOpType.mult,
                )
                if debug_taps and t == 0:
                    nc.sync.dma_start(out=dbg_xm[:, :], in_=xm[:])

                # transpose the 4 K-chunks of xm into one PSUM bank; separate
                # xT tiles per chunk (cast split across DVE/ACT)
                tp = tp_ps.tile([P, 4 * P], F32R, tag="tp")
                for k in range(NK):
                    nc.tensor.matmul(
                        out=tp[:, k * P : (k + 1) * P],
                        lhsT=xm[:, k * P : (k + 1) * P],
                        rhs=ident_sb[:],
                        is_transpose=True,
                        start=(k == 0), stop=(k == NK - 1),
                    )
                xTs = []
                for k in range(NK):
                    xT_k = xmt.tile([P, P], F32R, tag=f"xmT{k}")
                    nc.scalar.activation(
                        out=xT_k[:], in_=tp[:, k * P : (k + 1) * P],
                        func=mybir.ActivationFunctionType.Copy,
                    )
                    xTs.append(xT_k)
                if debug_taps and t == 0:
                    nc.sync.dma_start(out=dbg_xmt[:, :], in_=xTs[0][:])

                # one PSUM bank per expert; expert e needs chunks 0..ce-1
                banks = []
                for e in range(E):
                    ybank = y_ps.tile([P, D], F32, tag=f"yps{e}")
                    banks.append(ybank)
                for e in range(E):
                    ce = CHUNKS_PER_EXPERT[e]
                    for k in range(ce):
                        j = sum(CHUNKS_PER_EXPERT[:e]) + k
                        nc.tensor.matmul(
                            out=banks[e][:],
                            lhsT=xTs[k][:],
                            rhs=wt_sb[:, j, :],
                            start=(k == 0), stop=(k == ce - 1),
                        )
                # assemble: rows of expert e from bank e, then add bias row
                y_sb = work.tile([P, D], F32, tag="ysb")
                nc.scalar.activation(
                    out=y_sb[:], in_=banks[0][:],
                    func=mybir.ActivationFunctionType.Copy,
                    scale=ind_f[:, 0:1],
                )
                for e in range(1, E):
                    nc.vector.copy_predicated(
                        out=y_sb[:],
                        mask=ind_i8[:, e : e + 1].to_broadcast([P, D]),
                        data=banks[e][:],
                    )
                nc.vector.tensor_tensor(
                    out=y_sb[:], in0=y_sb[:], in1=emb_t[:, DB:],
                    op=mybir.AluOpType.add,
                )
                nc.sync.dma_start(out=y[t * P : (t + 1) * P, :], in_=y_sb[:])

    nc.compile()
    return nc


def prep_core_inputs(input_ids_row, emb_table, gate_table, expert_w, expert_b,
                     aug=None):
    """Host-side layout prep for one core. input_ids_row: (S,) int."""
    ids = np.ascontiguousarray(
        input_ids_row.reshape(NT, P).T.astype(np.int32)
    )  # [P, NT]: ids[p, t] = token t*128+p
    if aug is None:
        aug = build_aug_table(emb_table, gate_table, expert_b)
    # wt[p, j, :] = expert_w[e].T[128k+p, :] = expert_w[e][:, 128k+p] for j=(e,k)
    wt_full = np.transpose(expert_w, (2, 0, 1)).reshape(NK, P, E, D)  # [k,p,e,o]
    cols = []
    for e in range(E):
        for k in range(CHUNKS_PER_EXPERT[e]):
            cols.append(wt_full[k, :, e, :])  # [P, D]
    wt = np.ascontiguousarray(np.stack(cols, axis=1), dtype=np.float32)  # [P,NJ,D]
    iota = np.ascontiguousarray(
        np.broadcast_to(np.arange(D, dtype=np.float32), (P, D))
    )
    ident = np.eye(P, dtype=np.float32)
    iota5 = np.ascontiguousarray(
        np.broadcast_to(np.arange(E, dtype=np.float32), (P, E))
    )
    return {
        "ids": ids,
        "emb": aug,
        "wt": wt,
        "iota": iota,
        "ident": ident,
        "iota5": iota5,
    }


def build_aug_table(emb_table, gate_table, expert_b):
    """Weights-only preprocessing: per vocab row v append gate, expert index
    e(v) = clip(ceil(gate*512)//102, 0, 4), and the selected bias row."""
    g = gate_table[:, 0].astype(np.float32) * np.float32(D)
    count = np.ceil(g)
    eidx = np.clip((count // float(D // E)).astype(np.int64), 0, E - 1)
    aug = np.zeros((VOCAB, DA), dtype=np.float32)
    aug[:, :D] = emb_table
    aug[:, DG] = gate_table[:, 0]
    aug[:, DE] = eidx.astype(np.float32)
    aug[:, DB:] = expert_b[eidx]
    return aug


_CACHED_NC = None


def kernel(input_ids, emb_table, gate_table, expert_w, expert_b):
    global _CACHED_NC
    input_ids = np.asarray(input_ids)
    emb_table = np.asarray(emb_table, dtype=np.float32)
    gate_table = np.asarray(gate_table, dtype=np.float32)
    expert_w = np.asarray(expert_w, dtype=np.float32)
    expert_b = np.asarray(expert_b, dtype=np.float32)

    if _CACHED_NC is None:
        _CACHED_NC = build_program()
    nc = _CACHED_NC

    shared = None
    in_maps = []
    for c in range(B):
        m = prep_core_inputs(
            input_ids[c], emb_table, gate_table, expert_w, expert_b,
            aug=None if shared is None else shared["emb"],
        )
        if shared is None:
            shared = m
        else:
            # reuse identical replicated arrays across cores
            for k_ in ("emb", "wt", "iota", "ident", "iota5"):
                m[k_] = shared[k_]
        in_maps.append(m)

    trace = bool(int(os.environ.get("BASS_KERNEL_TRACE", "0")))
    res = bass_utils.run_bass_kernel_spmd(
        nc, in_maps, core_ids=list(range(B)), trace=trace
    )
    kernel.last_result = res
    out = np.stack([res.results[c]["y"] for c in range(B)], axis=0)
    return out.astype(np.float32)
